# revision 57
# baseline (speedup 1.0000x reference)
"""Multi-head attention (B=2, S=2048, D=1024, H=16, DH=64) on 8 TRN2 cores.

Sharding: core c handles batch b = c//4 and head group g = c%4 (4 heads).
Per core, for its (b, g):
    qhT/khT = per-head-pair projections in transposed layout [e, s] (bf16),
    vhe = projected V in natural [j, e] layout with a ones column per head,
    S^T = Kh @ Qh^T per head (keys j on partitions),
    P^T = exp(S^T / sqrt(dk)) -> bf16 SBUF tiles [j, i],
    PV (transposed): stationary = P^T chunk [128 j, 128 i], moving =
        vhe [128 j, 65] -> acc[i, e|den] accumulated over j-chunks in PSUM.
        The ones column makes acc[:, 64] the softmax denominator, which sits
        on the partition (i) axis so normalization is a per-partition
        tensor_scalar multiply on DVE.
    norm2 [i, e-pair] tiles are DMA-transposed (XBAR) into outT [e, i],
    PT_partial = Wf^T-slice @ outT  -> partial final projection [D, S],
        DMA'd directly from PSUM to HBM.
Host: out[b] = (sum_g PT_partial).T + bf.

All matmul inputs are bf16 (1 PE cycle/row); PSUM accumulation is fp32.
Multiple PSUM accumulation groups share a bank: the bank's first matmul uses
start=True (which zeroes the whole 2KB region), later groups start on
start=False over the zeroed space.

Schedule: K/V stream and project per 512-key block with head-0 attention
chasing them; remaining heads run ACT(exp)-bound with Q-ib1 projection and
the final projection tiles deferred into their PE slack.
"""

import sys

sys.path.insert(0, "/opt/trn_rl_repo")

from contextlib import ExitStack

import ml_dtypes
import numpy as np

import concourse.mybir as mybir
import concourse.tile as tile
from concourse import bacc
from concourse.bass_utils import run_bass_kernel_spmd

B, S, D, H, DH = 2, 2048, 1024, 16, 64
NCORES = 8
GPB = 4  # head-group cores per batch
HPG = H // GPB  # heads per group (4)
CW = HPG * DH  # concat width per core (256)
NPAIR = HPG // 2  # head pairs per group (2)
DCH = D // 128  # d chunks (8)
JCH = S // 128  # key chunks (16)
NSB = S // 512  # 512-wide key stream blocks (4)
IB = 1024  # i-block width for attention
NIB = S // IB  # 2
F32 = mybir.dt.float32
BF16 = mybir.dt.bfloat16
AF = mybir.ActivationFunctionType
INV_SQRT_DK = 1.0 / np.sqrt(DH)
NPBF16 = ml_dtypes.bfloat16

_CACHE = {}
PHASE_LOG = []  # (label, next_instruction_name) markers recorded during build




def _build():
    nc = bacc.Bacc("TRN2", target_bir_lowering=False, debug=False, num_devices=NCORES)

    qt_d = nc.dram_tensor("qt", [D, S], BF16, kind="ExternalInput").ap()
    kt_d = nc.dram_tensor("kt", [D, S], BF16, kind="ExternalInput").ap()
    vt_d = nc.dram_tensor("vt", [D, S], BF16, kind="ExternalInput").ap()
    wq_d = nc.dram_tensor("wq", [D, CW], BF16, kind="ExternalInput").ap()
    wk_d = nc.dram_tensor("wk", [D, CW], BF16, kind="ExternalInput").ap()
    wv_d = nc.dram_tensor("wv", [D, CW], BF16, kind="ExternalInput").ap()
    wf_d = nc.dram_tensor("wf", [CW, D], BF16, kind="ExternalInput").ap()
    bq_d = nc.dram_tensor("bq", [CW], F32, kind="ExternalInput").ap()
    bk_d = nc.dram_tensor("bk", [CW], F32, kind="ExternalInput").ap()
    bv_d = nc.dram_tensor("bv", [CW], F32, kind="ExternalInput").ap()
    ones_d = nc.dram_tensor("ones32", [128, 2 * JCH, 1], BF16, kind="ExternalInput").ap()
    onesr_d = nc.dram_tensor("ones_row", [1, 128], BF16, kind="ExternalInput").ap()
    ident_d = nc.dram_tensor("ident", [128, 128], BF16, kind="ExternalInput").ap()
    pt_d = [
        nc.dram_tensor(f"pt{cc}", [D, S], BF16, kind="ExternalOutput").ap()
        for cc in range(2)
    ]

    with (
        tile.TileContext(nc) as tc,
        nc.allow_low_precision(reason="bf16 matmul pipeline is intentional"),
        ExitStack() as ctx,
    ):
        const = ctx.enter_context(tc.tile_pool(name="const", bufs=1))
        persist = ctx.enter_context(tc.tile_pool(name="persist", bufs=1))

        wq_sb = const.tile([128, DCH * CW], BF16, tag="wq")
        wk_sb = const.tile([128, DCH * CW], BF16, tag="wk")
        wv_sb = const.tile([128, DCH * CW], BF16, tag="wv")
        wf_sb = const.tile([128, 2 * D], BF16, tag="wf")
        bq_sb = const.tile([128, NPAIR], F32, tag="bq")
        bk_sb = const.tile([128, NPAIR], F32, tag="bk")
        bv_sb = const.tile([128, NPAIR], F32, tag="bv")
        ones128 = const.tile([1, 128], BF16, tag="ones")
        ident_sb = const.tile([128, 128], BF16, tag="ident")
        ones32 = const.tile([128, 2 * JCH, 1], BF16, tag="ones32")

        def load_w(w_sb, w_dram):
            nc.sync.dma_start(
                out=w_sb[:].rearrange("p (c e) -> p c e", c=DCH),
                in_=w_dram.rearrange("(c p) e -> p c e", p=128),
            )

        def load_b(b_sb, b_dram):
            nc.sync.dma_start(out=b_sb[:], in_=b_dram.rearrange("(r p) -> p r", p=128))

        qhT = [persist.tile([128, S], BF16, tag=f"qhT{r}", name=f"qhT{r}") for r in range(NPAIR)]
        khT = [persist.tile([128, S], BF16, tag=f"khT{r}", name=f"khT{r}") for r in range(NPAIR)]
        vhe = [persist.tile([128, JCH * 130], BF16, tag=f"vhe{r}", name=f"vhe{r}") for r in range(NPAIR)]
        outT = [persist.tile([128, S], BF16, tag=f"outT{r}", name=f"outT{r}") for r in range(NPAIR)]

        def mark(label):
            PHASE_LOG.append((label, nc._state.get_next_instruction_name()))

        with (
            tc.tile_pool(name="qx", bufs=2) as qx_pool,
            tc.tile_pool(name="kx", bufs=4) as kx_pool,
            tc.tile_pool(name="vx", bufs=2) as vx_pool,
            tc.tile_pool(name="pexp", bufs=26) as pexp_pool,
            tc.tile_pool(name="rc", bufs=8) as rc_pool,
            tc.tile_pool(name="n2", bufs=8) as n2_pool,
            tc.tile_pool(name="fo", bufs=10) as fo_pool,
            tc.tile_pool(name="foq", bufs=3) as foq_pool,
            tc.tile_pool(name="ps_sc", bufs=2, space="PSUM") as ps_sc,
            tc.tile_pool(name="ps_acc", bufs=2, space="PSUM") as ps_acc,
            tc.tile_pool(name="ps_ms", bufs=2, space="PSUM") as ps_ms,
        ):
            # ---------- emitters ----------
            def emit_q_dmas(ib_, split=False):
                """Load the [D, IB] Q slice; optionally as two adjacent
                column-half DMAs so the first qproj half starts sooner."""
                t = qx_pool.tile([128, DCH, IB], BF16, tag="qx", name="qx")
                qsrc = qt_d.rearrange("(c p) s -> p c s", p=128)
                if split:
                    for h in range(2):
                        i0 = IB * ib_ + 512 * h
                        nc.sync.dma_start(
                            out=t[:, :, 512 * h : 512 * (h + 1)],
                            in_=qsrc[:, :, i0 : i0 + 512],
                        )
                else:
                    nc.sync.dma_start(
                        out=t[:], in_=qsrc[:, :, IB * ib_ : IB * (ib_ + 1)]
                    )
                return [t[:, d, :] for d in range(DCH)]

            def emit_kv_dmas(sblk, dram, pool, tag):
                t = pool.tile([128, DCH, 512], BF16, tag=tag, name=tag)
                nc.sync.dma_start(
                    out=t[:],
                    in_=dram.rearrange("(c p) s -> p c s", p=128)[
                        :, :, 512 * sblk : 512 * (sblk + 1)
                    ],
                )
                return [t[:, d, :] for d in range(DCH)]

            def emit_qproj_half(ib_, r, half, qx):
                """One 512-col half of the Q projection for pair r."""
                mark(f"qproj{ib_}r{r}h{half}")
                i0 = IB * ib_ + 512 * half
                ps_q = ps_ms.tile([128, 512], F32, tag="ms", name="ps_q")
                for d in range(DCH):
                    w_st = wq_sb[:, CW * d + 128 * r : CW * d + 128 * (r + 1)]
                    nc.tensor.matmul(
                        ps_q[:],
                        w_st,
                        qx[d][:, 512 * half : 512 * (half + 1)],
                        start=(d == 0),
                        stop=(d == DCH - 1),
                    )
                nc.vector.tensor_scalar_add(
                    qhT[r][:, i0 : i0 + 512], ps_q[:], bq_sb[:, r : r + 1]
                )

            def emit_kproj_pair(sblk, r, kx):
                mark(f"kproj{sblk}r{r}")
                ps_kb = ps_ms.tile([128, 512], F32, tag="ms", name="ps_kb")
                for d in range(DCH):
                    w_st = wk_sb[:, CW * d + 128 * r : CW * d + 128 * (r + 1)]
                    nc.tensor.matmul(
                        ps_kb[:],
                        w_st,
                        kx[d][:],
                        start=(d == 0),
                        stop=(d == DCH - 1),
                    )
                nc.vector.tensor_scalar_add(
                    khT[r][:, 512 * sblk : 512 * (sblk + 1)],
                    ps_kb[:],
                    bk_sb[:, r : r + 1],
                )

            def emit_vproj_jpair(j0, vx):
                """Project V for j-chunks j0, j0+1: two 256-col accumulation
                groups in one PSUM bank (the first group's start=True zeroes
                the whole bank region), then scatter into vhe."""
                mark(f"vproj{j0}")
                reg = ps_ms.tile([128, 512], F32, tag="ms", name="ps_vh")
                for jh in range(2):
                    j = j0 + jh
                    sub = reg[:, 256 * jh : 256 * (jh + 1)]
                    for d in range(DCH):
                        nc.tensor.matmul(
                            sub,
                            vx[d][:, 128 * (j % 4) : 128 * (j % 4 + 1)],
                            wv_sb[:, CW * d : CW * (d + 1)],
                            start=(d == 0 and jh == 0),
                            stop=(jh == 1 and d == DCH - 1),
                            skip_group_check=True,
                        )
                for jh in range(2):
                    j = j0 + jh
                    sub = reg[:, 256 * jh : 256 * (jh + 1)]
                    for r in range(NPAIR):
                        dst = vhe[r][:, 130 * j : 130 * j + 130]
                        nc.vector.tensor_copy(
                            dst.rearrange("p (b e) -> p b e", e=65)[:, :, 0:64],
                            sub[:, 128 * r : 128 * (r + 1)]
                            .rearrange("p (b e) -> p b e", e=64),
                        )

            # pending normalize state: (accA, accB, h, ib)
            pending_norm_box = [None]
            norm2_box = [[None] * 8, [None] * 8]  # per ib: 8 n2 tiles

            def emit_norm(pend, ics=range(8), transpose=True):
                accA, accB, h, ib_ = pend
                r, q = h // 2, h % 2
                mark(f"norm_h{h}i{ib_}")
                with nc.named_scope(f"norm_h{h}i{ib_}"):
                    for bk_i, acc in ((0, accA), (1, accB)):
                        bank_ics = [ic for ic in ics if (ic // 4) == bk_i]
                        if not bank_ics:
                            continue
                        rc = rc_pool.tile([128, 4], F32, tag="rc", name="rc")
                        nc.vector.reciprocal(
                            rc[:].rearrange("p (c w) -> p c w", w=1),
                            acc[:].rearrange("p (c w) -> p c w", w=128)[:, :, 64:65],
                        )
                        for ic in bank_ics:
                            off = 128 * (ic % 4)
                            if q == 0:
                                n2 = n2_pool.tile([128, 128], BF16, tag="n2", name="n2")
                                norm2_box[ib_][ic] = n2
                            else:
                                n2 = norm2_box[ib_][ic]
                            nc.vector.tensor_scalar_mul(
                                n2[:, 64 * q : 64 * (q + 1)],
                                acc[:, off : off + 64],
                                rc[:, (ic % 4) : (ic % 4) + 1],
                            )
                        if q == 1 and transpose:
                            tp = ps_ms.tile([128, 1024], BF16, tag="ms", name="tp")
                            for k, ic in enumerate(bank_ics):
                                nc.tensor.transpose(
                                    tp[:, 128 * k : 128 * (k + 1)],
                                    norm2_box[ib_][ic][:],
                                    ident_sb[:],
                                )
                            i0 = IB * ib_ + 128 * bank_ics[0]
                            nc.vector.tensor_scalar_add(
                                outT[r][:, i0 : i0 + 128 * len(bank_ics)],
                                tp[:, 0 : 128 * len(bank_ics)],
                                bv_sb[:, r : r + 1],
                            )

            def emit_final_half(cc, ib_, f, i4, copy_eng=None):
                """Partial final projection for pair cc only (host sums)."""
                mark(f"final{ib_}c{cc}f{f}i{i4}")
                i0 = IB * ib_ + 512 * i4
                pf = ps_ms.tile([128, 512], F32, tag="ms", name="pf")
                nc.tensor.matmul(
                    pf[:],
                    wf_sb[:, D * cc + 128 * f : D * cc + 128 * (f + 1)],
                    outT[cc][:, i0 : i0 + 512],
                    start=True,
                    stop=True,
                )
                fo = fo_pool.tile([128, 512], BF16, tag="fo", name="fo")
                if copy_eng == "act":
                    nc.scalar.copy(fo[:], pf[:])
                else:
                    nc.vector.tensor_copy(fo[:], pf[:])
                nc.sync.dma_start(
                    out=pt_d[cc][128 * f : 128 * (f + 1), i0 : i0 + 512], in_=fo[:]
                )

            def emit_pv(r, q, pexp, jc, accA, accB):
                vmov = vhe[r][:, 130 * jc + 65 * q : 130 * jc + 65 * (q + 1)]
                for ic in range(8):
                    tgt = accA if ic < 4 else accB
                    off = 128 * (ic % 4)
                    nc.tensor.matmul(
                        tgt[:, off : off + 65],
                        pexp[:, 128 * ic : 128 * (ic + 1)],
                        vmov,
                        start=(jc == 0 and ic % 4 == 0),
                        stop=(jc == JCH - 1),
                        skip_group_check=True,
                    )

            def emit_scores_exp(h, ib_, jc):
                """Scores + exp for head h; returns the pexp tile."""
                r, q = h // 2, h % 2
                mark(f"attn_h{h}i{ib_}jc{jc}")
                qs = slice(64 * q, 64 * (q + 1))
                s_ps = ps_sc.tile([128, IB], F32, tag="sc", name="s_ps")
                for k in range(IB // 512):
                    nc.tensor.matmul(
                        s_ps[:, 512 * k : 512 * (k + 1)],
                        khT[r][qs, 128 * jc : 128 * (jc + 1)],
                        qhT[r][qs, IB * ib_ + 512 * k : IB * ib_ + 512 * (k + 1)],
                        start=True,
                        stop=True,
                    )
                pexp = pexp_pool.tile([128, IB], BF16, tag="pexp", name="pexp")
                nc.scalar.activation(pexp[:], s_ps[:], AF.Exp, scale=INV_SQRT_DK)
                return pexp

            PVLAG = 6

            def emit_attn_jc(h, ib_, jc, accA, accB, prev_box):
                """Emit scores[jc] and exp[jc]; emit PV[jc-PVLAG] (software-
                pipelined so PV inputs are always long-ready)."""
                r, q = h // 2, h % 2
                qs = slice(64 * q, 64 * (q + 1))
                s_ps = ps_sc.tile([128, IB], F32, tag="sc", name="s_ps")
                mark(f"attn_h{h}i{ib_}jc{jc}")
                for k in range(IB // 512):
                    nc.tensor.matmul(
                        s_ps[:, 512 * k : 512 * (k + 1)],
                        khT[r][qs, 128 * jc : 128 * (jc + 1)],
                        qhT[r][qs, IB * ib_ + 512 * k : IB * ib_ + 512 * (k + 1)],
                        start=True,
                        stop=True,
                    )
                if len(prev_box) >= PVLAG:
                    emit_pv(r, q, *prev_box.pop(0), accA, accB)
                pexp = pexp_pool.tile([128, IB], BF16, tag="pexp", name="pexp")
                nc.scalar.activation(pexp[:], s_ps[:], AF.Exp, scale=INV_SQRT_DK)
                prev_box.append((pexp, jc))

            def emit_attention(h, ib_, deferred, defer_slots=(3, 5, 7, 9, 11, 13)):
                """Full attention for head h on i-block ib_. Emits the PREVIOUS
                head's normalize after jc 1 (its acc banks drain early)."""
                with nc.named_scope(f"attn{ib_}h{h}"):
                    accA = ps_acc.tile([128, 512], F32, tag="acc", name="accA")
                    accB = ps_acc.tile([128, 512], F32, tag="acc", name="accB")
                    prev_box = []
                    for jc in range(JCH):
                        emit_attn_jc(h, ib_, jc, accA, accB, prev_box)
                        if jc == 1 and pending_norm_box[0] is not None:
                            emit_norm(pending_norm_box[0])
                            pending_norm_box[0] = None
                        if jc in defer_slots and deferred:
                            deferred.pop(0)()
                    while prev_box:
                        emit_pv(h // 2, h % 2, *prev_box.pop(0), accA, accB)
                    pending_norm_box[0] = (accA, accB, h, ib_)
                while deferred:
                    deferred.pop(0)()

            # ---------- schedule ----------
            # prologue DMAs ordered so the chase's first consumers unblock in
            # the order the PE needs them: wk+K0 (kproj), wv+V0 (vproj),
            # wq+Q0 (qproj), biases/ones interleaved early (tiny)
            load_w(wk_sb, wk_d)
            kx_next = emit_kv_dmas(0, kt_d, kx_pool, "kx")
            load_w(wq_sb, wq_d)
            qx0 = emit_q_dmas(0)
            load_w(wv_sb, wv_d)
            vx_next = emit_kv_dmas(0, vt_d, vx_pool, "vx")
            load_b(bk_sb, bk_d)
            load_b(bq_sb, bq_d)
            load_b(bv_sb, bv_d)
            nc.sync.dma_start(out=ones128[:], in_=onesr_d)
            nc.sync.dma_start(out=ones32[:], in_=ones_d)
            nc.sync.dma_start(out=ident_sb[:], in_=ident_d)
            for r in range(NPAIR):
                nc.vector.tensor_copy(
                    vhe[r][:].rearrange("p (c w) -> p c w", w=65)[:, :, 64:65],
                    ones32[:],
                )

            # chase: per 512-j block: K/V dma (next), r0 K proj + V proj,
            # h0 attention. kproj r1 and qproj r1 are deferred into the
            # h1/h2 attention windows (ACT-bound there, PE-bound here).
            kx_all = [None] * NSB
            pexp_h1 = [None] * JCH
            with nc.named_scope("chase"):
                accA0 = ps_acc.tile([128, 512], F32, tag="acc", name="accA")
                accB0 = ps_acc.tile([128, 512], F32, tag="acc", name="accB")
                prev0 = []

                def chase_jc(jc):
                    emit_attn_jc(0, 0, jc, accA0, accB0, prev0)
                    pexp_h1[jc] = emit_scores_exp(1, 0, jc)

                for sblk in range(NSB):
                    kx, vx = kx_next, vx_next
                    kx_all[sblk] = kx
                    if sblk + 1 < NSB:
                        kx_next = emit_kv_dmas(sblk + 1, kt_d, kx_pool, "kx")
                        vx_next = emit_kv_dmas(sblk + 1, vt_d, vx_pool, "vx")
                    if sblk == 0:
                        emit_kproj_pair(sblk, 0, kx)
                        for half in range(2):
                            emit_qproj_half(0, 0, half, qx0)
                        emit_vproj_jpair(0, vx)
                        chase_jc(0)
                        chase_jc(1)
                        emit_vproj_jpair(2, vx)
                        chase_jc(2)
                        chase_jc(3)
                    else:
                        emit_kproj_pair(sblk, 0, kx)
                        emit_vproj_jpair(4 * sblk, vx)
                        chase_jc(4 * sblk + 0)
                        chase_jc(4 * sblk + 1)
                        emit_vproj_jpair(4 * sblk + 2, vx)
                        chase_jc(4 * sblk + 2)
                        chase_jc(4 * sblk + 3)
                    if sblk == 1:
                        emit_kproj_pair(0, 1, kx_all[0])
                    elif sblk == 2:
                        for half in range(2):
                            emit_qproj_half(0, 1, half, qx0)
                while prev0:
                    emit_pv(0, 0, *prev0.pop(0), accA0, accB0)
                pending_norm_box[0] = (accA0, accB0, 0, 0)

            # h0 norm releases the acc banks for h1's deferred PV burst,
            # which rides inside h2's unit so ACT never drains
            emit_norm(pending_norm_box[0])
            pending_norm_box[0] = None
            acc1_box = [None]

            def burst_half(lo, hi):
                with nc.named_scope("h1burst"):
                    if acc1_box[0] is None:
                        acc1_box[0] = (
                            ps_acc.tile([128, 512], F32, tag="acc", name="accA"),
                            ps_acc.tile([128, 512], F32, tag="acc", name="accB"),
                        )
                    accA1, accB1 = acc1_box[0]
                    for jc in range(lo, hi):
                        emit_pv(0, 1, pexp_h1[jc], jc, accA1, accB1)
                    if hi == JCH:
                        emit_norm((accA1, accB1, 1, 0))

            # steady phase: remaining 7 head-blocks, ACT(exp)-bound; defer the
            # leftover projections and ib0's final tiles into their PE slack
            qx1 = emit_q_dmas(1)
            nc.sync.dma_start(
                out=wf_sb[:].rearrange("p (c f) -> p c f", c=2),
                in_=wf_d.rearrange("(c p) f -> p c f", p=128),
            )
            def finals(cc, ib_, lo, hi, pair=False):
                fs = [
                    (lambda f=f, i4=i4: emit_final_half(cc, ib_, f, i4))
                    for i4 in range(IB // 512)
                    for f in range(D // 128)
                ][lo:hi]
                if not pair:
                    return fs
                return [
                    (lambda a=fs[i], b=(fs[i + 1] if i + 1 < len(fs) else None): (
                        a(), b() if b else None
                    ))
                    for i in range(0, len(fs), 2)
                ]

            d_h2i0 = [
                (lambda: emit_kproj_pair(1, 1, kx_all[1])),
                (lambda: burst_half(0, 8)),
                (lambda: burst_half(8, JCH)),
                (lambda: emit_kproj_pair(2, 1, kx_all[2])),
                (lambda: emit_kproj_pair(3, 1, kx_all[3])),
            ]
            d_h3i0 = [
                (lambda half=half: emit_qproj_half(1, 0, half, qx1))
                for half in range(2)
            ] + finals(0, 0, 0, 6)
            d_h0i1 = finals(0, 0, 6, 16) + finals(1, 0, 0, 4)
            d_h1i1 = [
                (lambda half=half: emit_qproj_half(1, 1, half, qx1))
                for half in range(2)
            ] + finals(1, 0, 4, 14)
            d_h2i1 = finals(1, 0, 14, 16) + finals(0, 1, 0, 8)
            d_h3i1 = finals(0, 1, 8, 16)
            emit_attention(2, 0, d_h2i0, defer_slots=(0, 3, 6, 9, 12))
            emit_attention(3, 0, d_h3i0, defer_slots=(1, 5, 9, 10, 11, 12, 13, 14))
            emit_attention(0, 1, d_h0i1, defer_slots=tuple(range(2, 16)))
            emit_attention(1, 1, d_h1i1, defer_slots=(2, 6) + tuple(range(8, 15)))
            emit_attention(2, 1, d_h2i1, defer_slots=tuple(range(2, 14)))
            emit_attention(3, 1, d_h3i1, defer_slots=(2, 3, 4, 5, 6, 7, 8, 9))
            # tail: h3-ib1's normalize split per 512-i block, with the final
            # projection tiles for each block released as soon as its
            # transposes complete
            pend = pending_norm_box[0]
            pending_norm_box[0] = None
            with nc.named_scope("final1"):
                for i4 in range(2):
                    emit_norm(pend, ics=range(4 * i4, 4 * i4 + 4))
                    i0 = IB + 512 * i4
                    for quad in range(2):
                        foq = foq_pool.tile([128, 4, 512], BF16, tag="foq", name="foq")
                        # pair (sc banks) + two singles (idle acc banks):
                        # 4 psum buffers rotating so copies never pace PE
                        f0 = 4 * quad
                        mark(f"final1c1f{f0}i{i4}")
                        pf = ps_sc.tile([128, IB], F32, tag="sc", name="pfp")
                        for n in range(2):
                            nc.tensor.matmul(
                                pf[:, 512 * n : 512 * (n + 1)],
                                wf_sb[:, D + 128 * (f0 + n) : D + 128 * (f0 + n + 1)],
                                outT[1][:, i0 : i0 + 512],
                                start=True,
                                stop=True,
                            )
                        eng0 = nc.scalar.copy if quad % 2 == 0 else nc.vector.tensor_copy
                        eng0(foq[:, 0:2, :], pf[:].rearrange("p (a w) -> p a w", a=2))
                        for n in range(2, 4):
                            f = 4 * quad + n
                            mark(f"final1c1f{f}i{i4}")
                            pfs = ps_acc.tile([128, 512], F32, tag="acc", name="pfs")
                            nc.tensor.matmul(
                                pfs[:],
                                wf_sb[:, D + 128 * f : D + 128 * (f + 1)],
                                outT[1][:, i0 : i0 + 512],
                                start=True,
                                stop=True,
                            )
                            if n % 2:
                                nc.vector.tensor_copy(foq[:, n, :], pfs[:])
                            else:
                                nc.scalar.copy(foq[:, n, :], pfs[:])
                        if i4 == 1 and quad == 1:
                            for pr in range(2):
                                nc.sync.dma_start(
                                    out=pt_d[1]
                                    .rearrange("(f p) s -> p f s", p=128)[
                                        :,
                                        4 + 2 * pr : 6 + 2 * pr,
                                        i0 : i0 + 512,
                                    ],
                                    in_=foq[:, 2 * pr : 2 * pr + 2, :],
                                )
                        else:
                            nc.sync.dma_start(
                                out=pt_d[1]
                                .rearrange("(f p) s -> p f s", p=128)[
                                    :, 4 * quad : 4 * quad + 4, i0 : i0 + 512
                                ],
                                in_=foq[:],
                            )

    nc.compile()
    return nc


def _get_nc():
    if "nc" not in _CACHE:
        _CACHE["nc"] = _build()
    return _CACHE["nc"]


def kernel(Q, K, V, Wq, bq, Wk, bk, Wv, bv, Wf, bf):
    Q, K, V = np.asarray(Q), np.asarray(K), np.asarray(V)
    Wq, Wk, Wv, Wf = (np.asarray(a) for a in (Wq, Wk, Wv, Wf))
    bq, bk, bv, bf = (np.asarray(a) for a in (bq, bk, bv, bf))

    nc = _get_nc()

    def tobf(x):
        return np.ascontiguousarray(x.astype(NPBF16))

    qt = [tobf(Q[b].T) for b in range(B)]
    kt = [tobf(K[b].T) for b in range(B)]
    vt = [tobf(V[b].T) for b in range(B)]
    wq_g = [tobf(Wq[HPG * g : HPG * (g + 1)].transpose(1, 0, 2).reshape(D, CW)) for g in range(GPB)]
    wk_g = [tobf(Wk[HPG * g : HPG * (g + 1)].transpose(1, 0, 2).reshape(D, CW)) for g in range(GPB)]
    wv_g = [tobf(Wv[HPG * g : HPG * (g + 1)].transpose(1, 0, 2).reshape(D, CW)) for g in range(GPB)]
    wf_g = [tobf(Wf[CW * g : CW * (g + 1), :]) for g in range(GPB)]
    bq_g = [np.ascontiguousarray(bq[HPG * g : HPG * (g + 1)].reshape(CW), np.float32) for g in range(GPB)]
    bk_g = [np.ascontiguousarray(bk[HPG * g : HPG * (g + 1)].reshape(CW), np.float32) for g in range(GPB)]
    bv_g = [np.ascontiguousarray(bv[HPG * g : HPG * (g + 1)].reshape(CW), np.float32) for g in range(GPB)]

    ones_col = np.ones((128, 2 * JCH, 1), NPBF16)
    ones_row = np.ones((1, 128), NPBF16)
    ident = np.eye(128, dtype=NPBF16)
    in_maps = []
    for c in range(NCORES):
        b, g = c // GPB, c % GPB
        in_maps.append(
            {
                "qt": qt[b], "kt": kt[b], "vt": vt[b],
                "wq": wq_g[g], "wk": wk_g[g], "wv": wv_g[g], "wf": wf_g[g],
                "bq": bq_g[g], "bk": bk_g[g], "bv": bv_g[g],
                "ones32": ones_col, "ones_row": ones_row, "ident": ident,
            }
        )

    res = run_bass_kernel_spmd(nc, in_maps, list(range(NCORES)))

    out = np.empty((B, S, D), np.float32)
    bf32 = bf.astype(np.float32)
    for b in range(B):
        acc = None
        for g in range(GPB):
            r = res.results[GPB * b + g]
            part = r["pt0"].astype(np.float32) + r["pt1"].astype(np.float32)
            acc = part if acc is None else acc + part
        out[b] = acc.T + bf32
    return out


# revision 58
# speedup vs baseline: 1.0199x; 1.0199x over previous
"""Multi-head attention (B=2, S=2048, D=1024, H=16, DH=64) on 8 TRN2 cores.

Sharding: core c handles batch b = c//4 and head group g = c%4 (4 heads).
Per core, for its (b, g):
    qhT/khT = per-head-pair projections in transposed layout [e, s] (bf16),
    vhe = projected V in natural [j, e] layout with a ones column per head,
    S^T = Kh @ Qh^T per head (keys j on partitions),
    P^T = exp(S^T / sqrt(dk)) -> bf16 SBUF tiles [j, i],
    PV (transposed): stationary = P^T chunk [128 j, 128 i], moving =
        vhe [128 j, 65] -> acc[i, e|den] accumulated over j-chunks in PSUM.
        The ones column makes acc[:, 64] the softmax denominator, which sits
        on the partition (i) axis so normalization is a per-partition
        tensor_scalar multiply on DVE.
    norm2 [i, e-pair] tiles are DMA-transposed (XBAR) into outT [e, i],
    PT_partial = Wf^T-slice @ outT  -> partial final projection [D, S],
        DMA'd directly from PSUM to HBM.
Host: out[b] = (sum_g PT_partial).T + bf.

All matmul inputs are bf16 (1 PE cycle/row); PSUM accumulation is fp32.
Multiple PSUM accumulation groups share a bank: the bank's first matmul uses
start=True (which zeroes the whole 2KB region), later groups start on
start=False over the zeroed space.

Schedule: K/V stream and project per 512-key block with head-0 attention
chasing them; remaining heads run ACT(exp)-bound with Q-ib1 projection and
the final projection tiles deferred into their PE slack.
"""

import sys

sys.path.insert(0, "/opt/trn_rl_repo")

from contextlib import ExitStack

import ml_dtypes
import numpy as np

import concourse.mybir as mybir
import concourse.tile as tile
from concourse import bacc
from concourse.bass_utils import run_bass_kernel_spmd

B, S, D, H, DH = 2, 2048, 1024, 16, 64
NCORES = 8
GPB = 4  # head-group cores per batch
HPG = H // GPB  # heads per group (4)
CW = HPG * DH  # concat width per core (256)
NPAIR = HPG // 2  # head pairs per group (2)
DCH = D // 128  # d chunks (8)
JCH = S // 128  # key chunks (16)
NSB = S // 512  # 512-wide key stream blocks (4)
IB = 1024  # i-block width for attention
NIB = S // IB  # 2
F32 = mybir.dt.float32
BF16 = mybir.dt.bfloat16
AF = mybir.ActivationFunctionType
INV_SQRT_DK = 1.0 / np.sqrt(DH)
NPBF16 = ml_dtypes.bfloat16

_CACHE = {}
PHASE_LOG = []  # (label, next_instruction_name) markers recorded during build




def _build():
    nc = bacc.Bacc("TRN2", target_bir_lowering=False, debug=False, num_devices=NCORES)

    qt_d = nc.dram_tensor("qt", [D, S], BF16, kind="ExternalInput").ap()
    kt_d = nc.dram_tensor("kt", [D, S], BF16, kind="ExternalInput").ap()
    vt_d = nc.dram_tensor("vt", [D, S], BF16, kind="ExternalInput").ap()
    wq_d = nc.dram_tensor("wq", [D, CW], BF16, kind="ExternalInput").ap()
    wk_d = nc.dram_tensor("wk", [D, CW], BF16, kind="ExternalInput").ap()
    wv_d = nc.dram_tensor("wv", [D, CW], BF16, kind="ExternalInput").ap()
    wf_d = nc.dram_tensor("wf", [CW, D], BF16, kind="ExternalInput").ap()
    bq_d = nc.dram_tensor("bq", [CW], F32, kind="ExternalInput").ap()
    bk_d = nc.dram_tensor("bk", [CW], F32, kind="ExternalInput").ap()
    bv_d = nc.dram_tensor("bv", [CW], F32, kind="ExternalInput").ap()
    ones_d = nc.dram_tensor("ones32", [128, 2 * JCH, 1], BF16, kind="ExternalInput").ap()
    onesr_d = nc.dram_tensor("ones_row", [1, 128], BF16, kind="ExternalInput").ap()
    ident_d = nc.dram_tensor("ident", [128, 128], BF16, kind="ExternalInput").ap()
    pt_d = [
        nc.dram_tensor(f"pt{cc}", [D, S], BF16, kind="ExternalOutput").ap()
        for cc in range(2)
    ]

    with (
        tile.TileContext(nc) as tc,
        nc.allow_low_precision(reason="bf16 matmul pipeline is intentional"),
        ExitStack() as ctx,
    ):
        const = ctx.enter_context(tc.tile_pool(name="const", bufs=1))
        persist = ctx.enter_context(tc.tile_pool(name="persist", bufs=1))

        wq_sb = const.tile([128, DCH * CW], BF16, tag="wq")
        wk_sb = const.tile([128, DCH * CW], BF16, tag="wk")
        wv_sb = const.tile([128, DCH * CW], BF16, tag="wv")
        wf_sb = const.tile([128, 2 * D], BF16, tag="wf")
        bq_sb = const.tile([128, NPAIR], F32, tag="bq")
        bk_sb = const.tile([128, NPAIR], F32, tag="bk")
        bv_sb = const.tile([128, NPAIR], F32, tag="bv")
        ones128 = const.tile([1, 128], BF16, tag="ones")
        ident_sb = const.tile([128, 128], BF16, tag="ident")
        ones32 = const.tile([128, 2 * JCH, 1], BF16, tag="ones32")

        def load_w(w_sb, w_dram):
            nc.sync.dma_start(
                out=w_sb[:].rearrange("p (c e) -> p c e", c=DCH),
                in_=w_dram.rearrange("(c p) e -> p c e", p=128),
            )

        def load_b(b_sb, b_dram):
            nc.sync.dma_start(out=b_sb[:], in_=b_dram.rearrange("(r p) -> p r", p=128))

        qhT = [persist.tile([128, S], BF16, tag=f"qhT{r}", name=f"qhT{r}") for r in range(NPAIR)]
        khT = [persist.tile([128, S], BF16, tag=f"khT{r}", name=f"khT{r}") for r in range(NPAIR)]
        vhe = [persist.tile([128, JCH * 130], BF16, tag=f"vhe{r}", name=f"vhe{r}") for r in range(NPAIR)]
        outT = [persist.tile([128, S], BF16, tag=f"outT{r}", name=f"outT{r}") for r in range(NPAIR)]

        def mark(label):
            PHASE_LOG.append((label, nc._state.get_next_instruction_name()))

        with (
            tc.tile_pool(name="qx", bufs=2) as qx_pool,
            tc.tile_pool(name="kx", bufs=4) as kx_pool,
            tc.tile_pool(name="vx", bufs=2) as vx_pool,
            tc.tile_pool(name="pexp", bufs=26) as pexp_pool,
            tc.tile_pool(name="rc", bufs=8) as rc_pool,
            tc.tile_pool(name="n2", bufs=8) as n2_pool,
            tc.tile_pool(name="fo", bufs=10) as fo_pool,
            tc.tile_pool(name="foq", bufs=3) as foq_pool,
            tc.tile_pool(name="ps_sc", bufs=2, space="PSUM") as ps_sc,
            tc.tile_pool(name="ps_acc", bufs=2, space="PSUM") as ps_acc,
            tc.tile_pool(name="ps_ms", bufs=2, space="PSUM") as ps_ms,
        ):
            # ---------- emitters ----------
            def emit_q_dmas(ib_, split=False):
                """Load the [D, IB] Q slice; optionally as two adjacent
                column-half DMAs so the first qproj half starts sooner."""
                t = qx_pool.tile([128, DCH, IB], BF16, tag="qx", name="qx")
                qsrc = qt_d.rearrange("(c p) s -> p c s", p=128)
                if split:
                    for h in range(2):
                        i0 = IB * ib_ + 512 * h
                        nc.sync.dma_start(
                            out=t[:, :, 512 * h : 512 * (h + 1)],
                            in_=qsrc[:, :, i0 : i0 + 512],
                        )
                else:
                    nc.sync.dma_start(
                        out=t[:], in_=qsrc[:, :, IB * ib_ : IB * (ib_ + 1)]
                    )
                return [t[:, d, :] for d in range(DCH)]

            def emit_kv_dmas(sblk, dram, pool, tag):
                t = pool.tile([128, DCH, 512], BF16, tag=tag, name=tag)
                nc.sync.dma_start(
                    out=t[:],
                    in_=dram.rearrange("(c p) s -> p c s", p=128)[
                        :, :, 512 * sblk : 512 * (sblk + 1)
                    ],
                )
                return [t[:, d, :] for d in range(DCH)]

            def emit_qproj_half(ib_, r, half, qx):
                """One 512-col half of the Q projection for pair r."""
                mark(f"qproj{ib_}r{r}h{half}")
                i0 = IB * ib_ + 512 * half
                ps_q = ps_ms.tile([128, 512], F32, tag="ms", name="ps_q")
                for d in range(DCH):
                    w_st = wq_sb[:, CW * d + 128 * r : CW * d + 128 * (r + 1)]
                    nc.tensor.matmul(
                        ps_q[:],
                        w_st,
                        qx[d][:, 512 * half : 512 * (half + 1)],
                        start=(d == 0),
                        stop=(d == DCH - 1),
                    )
                nc.vector.tensor_scalar_add(
                    qhT[r][:, i0 : i0 + 512], ps_q[:], bq_sb[:, r : r + 1]
                )

            def emit_kproj_pair(sblk, r, kx):
                mark(f"kproj{sblk}r{r}")
                ps_kb = ps_ms.tile([128, 512], F32, tag="ms", name="ps_kb")
                for d in range(DCH):
                    w_st = wk_sb[:, CW * d + 128 * r : CW * d + 128 * (r + 1)]
                    nc.tensor.matmul(
                        ps_kb[:],
                        w_st,
                        kx[d][:],
                        start=(d == 0),
                        stop=(d == DCH - 1),
                    )
                nc.vector.tensor_scalar_add(
                    khT[r][:, 512 * sblk : 512 * (sblk + 1)],
                    ps_kb[:],
                    bk_sb[:, r : r + 1],
                )

            def emit_vproj_jpair(j0, vx):
                """Project V for j-chunks j0, j0+1: two 256-col accumulation
                groups in one PSUM bank (the first group's start=True zeroes
                the whole bank region), then scatter into vhe."""
                mark(f"vproj{j0}")
                reg = ps_ms.tile([128, 512], F32, tag="ms", name="ps_vh")
                for jh in range(2):
                    j = j0 + jh
                    sub = reg[:, 256 * jh : 256 * (jh + 1)]
                    for d in range(DCH):
                        nc.tensor.matmul(
                            sub,
                            vx[d][:, 128 * (j % 4) : 128 * (j % 4 + 1)],
                            wv_sb[:, CW * d : CW * (d + 1)],
                            start=(d == 0 and jh == 0),
                            stop=(jh == 1 and d == DCH - 1),
                            skip_group_check=True,
                        )
                for jh in range(2):
                    j = j0 + jh
                    sub = reg[:, 256 * jh : 256 * (jh + 1)]
                    for r in range(NPAIR):
                        dst = vhe[r][:, 130 * j : 130 * j + 130]
                        nc.vector.tensor_copy(
                            dst.rearrange("p (b e) -> p b e", e=65)[:, :, 0:64],
                            sub[:, 128 * r : 128 * (r + 1)]
                            .rearrange("p (b e) -> p b e", e=64),
                        )

            # pending normalize state: (accA, accB, h, ib)
            pending_norm_box = [None]
            norm2_box = [[None] * 8, [None] * 8]  # per ib: 8 n2 tiles

            def emit_norm(pend, ics=range(8), transpose=True):
                accA, accB, h, ib_ = pend
                r, q = h // 2, h % 2
                mark(f"norm_h{h}i{ib_}")
                with nc.named_scope(f"norm_h{h}i{ib_}"):
                    for bk_i, acc in ((0, accA), (1, accB)):
                        bank_ics = [ic for ic in ics if (ic // 4) == bk_i]
                        if not bank_ics:
                            continue
                        rc = rc_pool.tile([128, 4], F32, tag="rc", name="rc")
                        nc.vector.reciprocal(
                            rc[:].rearrange("p (c w) -> p c w", w=1),
                            acc[:].rearrange("p (c w) -> p c w", w=128)[:, :, 64:65],
                        )
                        for ic in bank_ics:
                            off = 128 * (ic % 4)
                            if q == 0:
                                n2 = n2_pool.tile([128, 128], BF16, tag="n2", name="n2")
                                norm2_box[ib_][ic] = n2
                            else:
                                n2 = norm2_box[ib_][ic]
                            nc.vector.tensor_scalar_mul(
                                n2[:, 64 * q : 64 * (q + 1)],
                                acc[:, off : off + 64],
                                rc[:, (ic % 4) : (ic % 4) + 1],
                            )
                        if q == 1 and transpose:
                            tp = ps_ms.tile([128, 1024], BF16, tag="ms", name="tp")
                            for k, ic in enumerate(bank_ics):
                                nc.tensor.transpose(
                                    tp[:, 128 * k : 128 * (k + 1)],
                                    norm2_box[ib_][ic][:],
                                    ident_sb[:],
                                )
                            i0 = IB * ib_ + 128 * bank_ics[0]
                            nc.vector.tensor_scalar_add(
                                outT[r][:, i0 : i0 + 128 * len(bank_ics)],
                                tp[:, 0 : 128 * len(bank_ics)],
                                bv_sb[:, r : r + 1],
                            )

            def emit_final_half(cc, ib_, f, i4, copy_eng=None):
                """Partial final projection for pair cc only (host sums)."""
                mark(f"final{ib_}c{cc}f{f}i{i4}")
                i0 = IB * ib_ + 512 * i4
                pf = ps_ms.tile([128, 512], F32, tag="ms", name="pf")
                nc.tensor.matmul(
                    pf[:],
                    wf_sb[:, D * cc + 128 * f : D * cc + 128 * (f + 1)],
                    outT[cc][:, i0 : i0 + 512],
                    start=True,
                    stop=True,
                )
                fo = fo_pool.tile([128, 512], BF16, tag="fo", name="fo")
                if copy_eng == "act":
                    nc.scalar.copy(fo[:], pf[:])
                else:
                    nc.vector.tensor_copy(fo[:], pf[:])
                nc.sync.dma_start(
                    out=pt_d[cc][128 * f : 128 * (f + 1), i0 : i0 + 512], in_=fo[:]
                )

            def emit_pv(r, q, pexp, jc, accA, accB):
                vmov = vhe[r][:, 130 * jc + 65 * q : 130 * jc + 65 * (q + 1)]
                for ic in range(8):
                    tgt = accA if ic < 4 else accB
                    off = 128 * (ic % 4)
                    nc.tensor.matmul(
                        tgt[:, off : off + 65],
                        pexp[:, 128 * ic : 128 * (ic + 1)],
                        vmov,
                        start=(jc == 0 and ic % 4 == 0),
                        stop=(jc == JCH - 1),
                        skip_group_check=True,
                    )

            def emit_scores_exp(h, ib_, jc):
                """Scores + exp for head h; returns the pexp tile."""
                r, q = h // 2, h % 2
                mark(f"attn_h{h}i{ib_}jc{jc}")
                qs = slice(64 * q, 64 * (q + 1))
                s_ps = ps_sc.tile([128, IB], F32, tag="sc", name="s_ps")
                for k in range(IB // 512):
                    nc.tensor.matmul(
                        s_ps[:, 512 * k : 512 * (k + 1)],
                        khT[r][qs, 128 * jc : 128 * (jc + 1)],
                        qhT[r][qs, IB * ib_ + 512 * k : IB * ib_ + 512 * (k + 1)],
                        start=True,
                        stop=True,
                    )
                pexp = pexp_pool.tile([128, IB], BF16, tag="pexp", name="pexp")
                nc.scalar.activation(pexp[:], s_ps[:], AF.Exp, scale=INV_SQRT_DK)
                return pexp

            PVLAG = 6

            def emit_attn_jc(h, ib_, jc, accA, accB, prev_box):
                """Emit scores[jc] and exp[jc]; emit PV[jc-PVLAG] (software-
                pipelined so PV inputs are always long-ready)."""
                r, q = h // 2, h % 2
                qs = slice(64 * q, 64 * (q + 1))
                s_ps = ps_sc.tile([128, IB], F32, tag="sc", name="s_ps")
                mark(f"attn_h{h}i{ib_}jc{jc}")
                for k in range(IB // 512):
                    nc.tensor.matmul(
                        s_ps[:, 512 * k : 512 * (k + 1)],
                        khT[r][qs, 128 * jc : 128 * (jc + 1)],
                        qhT[r][qs, IB * ib_ + 512 * k : IB * ib_ + 512 * (k + 1)],
                        start=True,
                        stop=True,
                    )
                if len(prev_box) >= PVLAG:
                    emit_pv(r, q, *prev_box.pop(0), accA, accB)
                pexp = pexp_pool.tile([128, IB], BF16, tag="pexp", name="pexp")
                nc.scalar.activation(pexp[:], s_ps[:], AF.Exp, scale=INV_SQRT_DK)
                prev_box.append((pexp, jc))

            def emit_attention(h, ib_, deferred, defer_slots=(3, 5, 7, 9, 11, 13)):
                """Full attention for head h on i-block ib_. Emits the PREVIOUS
                head's normalize after jc 1 (its acc banks drain early)."""
                with nc.named_scope(f"attn{ib_}h{h}"):
                    accA = ps_acc.tile([128, 512], F32, tag="acc", name="accA")
                    accB = ps_acc.tile([128, 512], F32, tag="acc", name="accB")
                    prev_box = []
                    for jc in range(JCH):
                        emit_attn_jc(h, ib_, jc, accA, accB, prev_box)
                        if jc == 1 and pending_norm_box[0] is not None:
                            emit_norm(pending_norm_box[0])
                            pending_norm_box[0] = None
                        if jc in defer_slots and deferred:
                            deferred.pop(0)()
                    while prev_box:
                        emit_pv(h // 2, h % 2, *prev_box.pop(0), accA, accB)
                    pending_norm_box[0] = (accA, accB, h, ib_)
                while deferred:
                    deferred.pop(0)()

            # ---------- schedule ----------
            # prologue DMAs ordered so the chase's first consumers unblock in
            # the order the PE needs them: wk+K0 (kproj), wv+V0 (vproj),
            # wq+Q0 (qproj), biases/ones interleaved early (tiny)
            load_w(wk_sb, wk_d)
            kx_next = emit_kv_dmas(0, kt_d, kx_pool, "kx")
            load_w(wv_sb, wv_d)
            vx_next = emit_kv_dmas(0, vt_d, vx_pool, "vx")
            load_w(wq_sb, wq_d)
            qx0 = emit_q_dmas(0)
            load_b(bk_sb, bk_d)
            load_b(bq_sb, bq_d)
            load_b(bv_sb, bv_d)
            nc.sync.dma_start(out=ones128[:], in_=onesr_d)
            nc.sync.dma_start(out=ones32[:], in_=ones_d)
            nc.sync.dma_start(out=ident_sb[:], in_=ident_d)
            for r in range(NPAIR):
                nc.vector.tensor_copy(
                    vhe[r][:].rearrange("p (c w) -> p c w", w=65)[:, :, 64:65],
                    ones32[:],
                )

            # chase: per 512-j block: K/V dma (next), r0 K proj + V proj,
            # h0 attention. kproj r1 and qproj r1 are deferred into the
            # h1/h2 attention windows (ACT-bound there, PE-bound here).
            kx_all = [None] * NSB
            pexp_h1 = [None] * JCH
            with nc.named_scope("chase"):
                accA0 = ps_acc.tile([128, 512], F32, tag="acc", name="accA")
                accB0 = ps_acc.tile([128, 512], F32, tag="acc", name="accB")
                prev0 = []

                def chase_jc(jc):
                    emit_attn_jc(0, 0, jc, accA0, accB0, prev0)
                    pexp_h1[jc] = emit_scores_exp(1, 0, jc)

                for sblk in range(NSB):
                    kx, vx = kx_next, vx_next
                    kx_all[sblk] = kx
                    if sblk + 1 < NSB:
                        kx_next = emit_kv_dmas(sblk + 1, kt_d, kx_pool, "kx")
                        vx_next = emit_kv_dmas(sblk + 1, vt_d, vx_pool, "vx")
                    if sblk == 0:
                        emit_kproj_pair(sblk, 0, kx)
                        emit_vproj_jpair(0, vx)
                        emit_vproj_jpair(2, vx)
                        for half in range(2):
                            emit_qproj_half(0, 0, half, qx0)
                        chase_jc(0)
                        chase_jc(1)
                        chase_jc(2)
                        chase_jc(3)
                    else:
                        emit_kproj_pair(sblk, 0, kx)
                        emit_vproj_jpair(4 * sblk, vx)
                        chase_jc(4 * sblk + 0)
                        chase_jc(4 * sblk + 1)
                        emit_vproj_jpair(4 * sblk + 2, vx)
                        chase_jc(4 * sblk + 2)
                        chase_jc(4 * sblk + 3)
                    if sblk == 1:
                        emit_kproj_pair(0, 1, kx_all[0])
                    elif sblk == 2:
                        for half in range(2):
                            emit_qproj_half(0, 1, half, qx0)
                while prev0:
                    emit_pv(0, 0, *prev0.pop(0), accA0, accB0)
                pending_norm_box[0] = (accA0, accB0, 0, 0)

            # h0 norm releases the acc banks for h1's deferred PV burst,
            # which rides inside h2's unit so ACT never drains
            emit_norm(pending_norm_box[0])
            pending_norm_box[0] = None
            acc1_box = [None]

            def burst_half(lo, hi):
                with nc.named_scope("h1burst"):
                    if acc1_box[0] is None:
                        acc1_box[0] = (
                            ps_acc.tile([128, 512], F32, tag="acc", name="accA"),
                            ps_acc.tile([128, 512], F32, tag="acc", name="accB"),
                        )
                    accA1, accB1 = acc1_box[0]
                    for jc in range(lo, hi):
                        emit_pv(0, 1, pexp_h1[jc], jc, accA1, accB1)
                    if hi == JCH:
                        emit_norm((accA1, accB1, 1, 0))

            # steady phase: remaining 7 head-blocks, ACT(exp)-bound; defer the
            # leftover projections and ib0's final tiles into their PE slack
            qx1 = emit_q_dmas(1)
            nc.sync.dma_start(
                out=wf_sb[:].rearrange("p (c f) -> p c f", c=2),
                in_=wf_d.rearrange("(c p) f -> p c f", p=128),
            )
            def finals(cc, ib_, lo, hi, pair=False):
                fs = [
                    (lambda f=f, i4=i4: emit_final_half(cc, ib_, f, i4))
                    for i4 in range(IB // 512)
                    for f in range(D // 128)
                ][lo:hi]
                if not pair:
                    return fs
                return [
                    (lambda a=fs[i], b=(fs[i + 1] if i + 1 < len(fs) else None): (
                        a(), b() if b else None
                    ))
                    for i in range(0, len(fs), 2)
                ]

            d_h2i0 = [
                (lambda: emit_kproj_pair(1, 1, kx_all[1])),
                (lambda: burst_half(0, 8)),
                (lambda: burst_half(8, JCH)),
                (lambda: emit_kproj_pair(2, 1, kx_all[2])),
                (lambda: emit_kproj_pair(3, 1, kx_all[3])),
            ]
            d_h3i0 = [
                (lambda half=half: emit_qproj_half(1, 0, half, qx1))
                for half in range(2)
            ] + finals(0, 0, 0, 6)
            d_h0i1 = finals(0, 0, 6, 16) + finals(1, 0, 0, 4)
            d_h1i1 = [
                (lambda half=half: emit_qproj_half(1, 1, half, qx1))
                for half in range(2)
            ] + finals(1, 0, 4, 14)
            d_h2i1 = finals(1, 0, 14, 16) + finals(0, 1, 0, 8)
            d_h3i1 = finals(0, 1, 8, 16)
            emit_attention(2, 0, d_h2i0, defer_slots=(0, 3, 6, 9, 12))
            emit_attention(3, 0, d_h3i0, defer_slots=(1, 5, 9, 10, 11, 12, 13, 14))
            emit_attention(0, 1, d_h0i1, defer_slots=tuple(range(2, 16)))
            emit_attention(1, 1, d_h1i1, defer_slots=(2, 6) + tuple(range(8, 15)))
            emit_attention(2, 1, d_h2i1, defer_slots=tuple(range(2, 14)))
            emit_attention(3, 1, d_h3i1, defer_slots=(2, 3, 4, 5, 6, 7, 8, 9))
            # tail: h3-ib1's normalize split per 512-i block, with the final
            # projection tiles for each block released as soon as its
            # transposes complete
            pend = pending_norm_box[0]
            pending_norm_box[0] = None
            with nc.named_scope("final1"):
                for i4 in range(2):
                    emit_norm(pend, ics=range(4 * i4, 4 * i4 + 4))
                    i0 = IB + 512 * i4
                    for quad in range(2):
                        foq = foq_pool.tile([128, 4, 512], BF16, tag="foq", name="foq")
                        # pair (sc banks) + two singles (idle acc banks):
                        # 4 psum buffers rotating so copies never pace PE
                        f0 = 4 * quad
                        mark(f"final1c1f{f0}i{i4}")
                        pf = ps_sc.tile([128, IB], F32, tag="sc", name="pfp")
                        for n in range(2):
                            nc.tensor.matmul(
                                pf[:, 512 * n : 512 * (n + 1)],
                                wf_sb[:, D + 128 * (f0 + n) : D + 128 * (f0 + n + 1)],
                                outT[1][:, i0 : i0 + 512],
                                start=True,
                                stop=True,
                            )
                        eng0 = nc.scalar.copy if quad % 2 == 0 else nc.vector.tensor_copy
                        eng0(foq[:, 0:2, :], pf[:].rearrange("p (a w) -> p a w", a=2))
                        for n in range(2, 4):
                            f = 4 * quad + n
                            mark(f"final1c1f{f}i{i4}")
                            pfs = ps_acc.tile([128, 512], F32, tag="acc", name="pfs")
                            nc.tensor.matmul(
                                pfs[:],
                                wf_sb[:, D + 128 * f : D + 128 * (f + 1)],
                                outT[1][:, i0 : i0 + 512],
                                start=True,
                                stop=True,
                            )
                            if n % 2:
                                nc.vector.tensor_copy(foq[:, n, :], pfs[:])
                            else:
                                nc.scalar.copy(foq[:, n, :], pfs[:])
                        if i4 == 1 and quad == 1:
                            for pr in range(2):
                                nc.sync.dma_start(
                                    out=pt_d[1]
                                    .rearrange("(f p) s -> p f s", p=128)[
                                        :,
                                        4 + 2 * pr : 6 + 2 * pr,
                                        i0 : i0 + 512,
                                    ],
                                    in_=foq[:, 2 * pr : 2 * pr + 2, :],
                                )
                        else:
                            nc.sync.dma_start(
                                out=pt_d[1]
                                .rearrange("(f p) s -> p f s", p=128)[
                                    :, 4 * quad : 4 * quad + 4, i0 : i0 + 512
                                ],
                                in_=foq[:],
                            )

    nc.compile()
    return nc


def _get_nc():
    if "nc" not in _CACHE:
        _CACHE["nc"] = _build()
    return _CACHE["nc"]


def kernel(Q, K, V, Wq, bq, Wk, bk, Wv, bv, Wf, bf):
    Q, K, V = np.asarray(Q), np.asarray(K), np.asarray(V)
    Wq, Wk, Wv, Wf = (np.asarray(a) for a in (Wq, Wk, Wv, Wf))
    bq, bk, bv, bf = (np.asarray(a) for a in (bq, bk, bv, bf))

    nc = _get_nc()

    def tobf(x):
        return np.ascontiguousarray(x.astype(NPBF16))

    qt = [tobf(Q[b].T) for b in range(B)]
    kt = [tobf(K[b].T) for b in range(B)]
    vt = [tobf(V[b].T) for b in range(B)]
    wq_g = [tobf(Wq[HPG * g : HPG * (g + 1)].transpose(1, 0, 2).reshape(D, CW)) for g in range(GPB)]
    wk_g = [tobf(Wk[HPG * g : HPG * (g + 1)].transpose(1, 0, 2).reshape(D, CW)) for g in range(GPB)]
    wv_g = [tobf(Wv[HPG * g : HPG * (g + 1)].transpose(1, 0, 2).reshape(D, CW)) for g in range(GPB)]
    wf_g = [tobf(Wf[CW * g : CW * (g + 1), :]) for g in range(GPB)]
    bq_g = [np.ascontiguousarray(bq[HPG * g : HPG * (g + 1)].reshape(CW), np.float32) for g in range(GPB)]
    bk_g = [np.ascontiguousarray(bk[HPG * g : HPG * (g + 1)].reshape(CW), np.float32) for g in range(GPB)]
    bv_g = [np.ascontiguousarray(bv[HPG * g : HPG * (g + 1)].reshape(CW), np.float32) for g in range(GPB)]

    ones_col = np.ones((128, 2 * JCH, 1), NPBF16)
    ones_row = np.ones((1, 128), NPBF16)
    ident = np.eye(128, dtype=NPBF16)
    in_maps = []
    for c in range(NCORES):
        b, g = c // GPB, c % GPB
        in_maps.append(
            {
                "qt": qt[b], "kt": kt[b], "vt": vt[b],
                "wq": wq_g[g], "wk": wk_g[g], "wv": wv_g[g], "wf": wf_g[g],
                "bq": bq_g[g], "bk": bk_g[g], "bv": bv_g[g],
                "ones32": ones_col, "ones_row": ones_row, "ident": ident,
            }
        )

    res = run_bass_kernel_spmd(nc, in_maps, list(range(NCORES)))

    out = np.empty((B, S, D), np.float32)
    bf32 = bf.astype(np.float32)
    for b in range(B):
        acc = None
        for g in range(GPB):
            r = res.results[GPB * b + g]
            part = r["pt0"].astype(np.float32) + r["pt1"].astype(np.float32)
            acc = part if acc is None else acc + part
        out[b] = acc.T + bf32
    return out


# revision 59
# speedup vs baseline: 1.0240x; 1.0040x over previous
"""Multi-head attention (B=2, S=2048, D=1024, H=16, DH=64) on 8 TRN2 cores.

Sharding: core c handles batch b = c//4 and head group g = c%4 (4 heads).
Per core, for its (b, g):
    qhT/khT = per-head-pair projections in transposed layout [e, s] (bf16),
    vhe = projected V in natural [j, e] layout with a ones column per head,
    S^T = Kh @ Qh^T per head (keys j on partitions),
    P^T = exp(S^T / sqrt(dk)) -> bf16 SBUF tiles [j, i],
    PV (transposed): stationary = P^T chunk [128 j, 128 i], moving =
        vhe [128 j, 65] -> acc[i, e|den] accumulated over j-chunks in PSUM.
        The ones column makes acc[:, 64] the softmax denominator, which sits
        on the partition (i) axis so normalization is a per-partition
        tensor_scalar multiply on DVE.
    norm2 [i, e-pair] tiles are DMA-transposed (XBAR) into outT [e, i],
    PT_partial = Wf^T-slice @ outT  -> partial final projection [D, S],
        DMA'd directly from PSUM to HBM.
Host: out[b] = (sum_g PT_partial).T + bf.

All matmul inputs are bf16 (1 PE cycle/row); PSUM accumulation is fp32.
Multiple PSUM accumulation groups share a bank: the bank's first matmul uses
start=True (which zeroes the whole 2KB region), later groups start on
start=False over the zeroed space.

Schedule: K/V stream and project per 512-key block with head-0 attention
chasing them; remaining heads run ACT(exp)-bound with Q-ib1 projection and
the final projection tiles deferred into their PE slack.
"""

import sys

sys.path.insert(0, "/opt/trn_rl_repo")

from contextlib import ExitStack

import ml_dtypes
import numpy as np

import concourse.mybir as mybir
import concourse.tile as tile
from concourse import bacc
from concourse.bass_utils import run_bass_kernel_spmd

B, S, D, H, DH = 2, 2048, 1024, 16, 64
NCORES = 8
GPB = 4  # head-group cores per batch
HPG = H // GPB  # heads per group (4)
CW = HPG * DH  # concat width per core (256)
NPAIR = HPG // 2  # head pairs per group (2)
DCH = D // 128  # d chunks (8)
JCH = S // 128  # key chunks (16)
NSB = S // 512  # 512-wide key stream blocks (4)
IB = 1024  # i-block width for attention
NIB = S // IB  # 2
F32 = mybir.dt.float32
BF16 = mybir.dt.bfloat16
AF = mybir.ActivationFunctionType
INV_SQRT_DK = 1.0 / np.sqrt(DH)
NPBF16 = ml_dtypes.bfloat16

_CACHE = {}
PHASE_LOG = []  # (label, next_instruction_name) markers recorded during build




def _build():
    nc = bacc.Bacc("TRN2", target_bir_lowering=False, debug=False, num_devices=NCORES)

    qt_d = nc.dram_tensor("qt", [D, S], BF16, kind="ExternalInput").ap()
    kt_d = nc.dram_tensor("kt", [D, S], BF16, kind="ExternalInput").ap()
    vt_d = nc.dram_tensor("vt", [D, S], BF16, kind="ExternalInput").ap()
    wq_d = nc.dram_tensor("wq", [D, CW], BF16, kind="ExternalInput").ap()
    wk_d = nc.dram_tensor("wk", [D, CW], BF16, kind="ExternalInput").ap()
    wv_d = nc.dram_tensor("wv", [D, CW], BF16, kind="ExternalInput").ap()
    wf_d = nc.dram_tensor("wf", [CW, D], BF16, kind="ExternalInput").ap()
    bq_d = nc.dram_tensor("bq", [CW], F32, kind="ExternalInput").ap()
    bk_d = nc.dram_tensor("bk", [CW], F32, kind="ExternalInput").ap()
    bv_d = nc.dram_tensor("bv", [CW], F32, kind="ExternalInput").ap()
    ones_d = nc.dram_tensor("ones32", [128, 2 * JCH, 1], BF16, kind="ExternalInput").ap()
    onesr_d = nc.dram_tensor("ones_row", [1, 128], BF16, kind="ExternalInput").ap()
    ident_d = nc.dram_tensor("ident", [128, 128], BF16, kind="ExternalInput").ap()
    pt_d = [
        nc.dram_tensor(f"pt{cc}", [D, S], BF16, kind="ExternalOutput").ap()
        for cc in range(2)
    ]

    with (
        tile.TileContext(nc) as tc,
        nc.allow_low_precision(reason="bf16 matmul pipeline is intentional"),
        ExitStack() as ctx,
    ):
        const = ctx.enter_context(tc.tile_pool(name="const", bufs=1))
        persist = ctx.enter_context(tc.tile_pool(name="persist", bufs=1))

        wq_sb = const.tile([128, DCH * CW], BF16, tag="wq")
        wk_sb = const.tile([128, DCH * CW], BF16, tag="wk")
        wv_sb = const.tile([128, DCH * CW], BF16, tag="wv")
        wf_sb = const.tile([128, 2 * D], BF16, tag="wf")
        bq_sb = const.tile([128, NPAIR], F32, tag="bq")
        bk_sb = const.tile([128, NPAIR], F32, tag="bk")
        bv_sb = const.tile([128, NPAIR], F32, tag="bv")
        ones128 = const.tile([1, 128], BF16, tag="ones")
        ident_sb = const.tile([128, 128], BF16, tag="ident")
        ones32 = const.tile([128, 2 * JCH, 1], BF16, tag="ones32")

        def load_w(w_sb, w_dram):
            nc.sync.dma_start(
                out=w_sb[:].rearrange("p (c e) -> p c e", c=DCH),
                in_=w_dram.rearrange("(c p) e -> p c e", p=128),
            )

        def load_b(b_sb, b_dram):
            nc.sync.dma_start(out=b_sb[:], in_=b_dram.rearrange("(r p) -> p r", p=128))

        qhT = [persist.tile([128, S], BF16, tag=f"qhT{r}", name=f"qhT{r}") for r in range(NPAIR)]
        khT = [persist.tile([128, S], BF16, tag=f"khT{r}", name=f"khT{r}") for r in range(NPAIR)]
        vhe = [persist.tile([128, JCH * 130], BF16, tag=f"vhe{r}", name=f"vhe{r}") for r in range(NPAIR)]
        outT = [persist.tile([128, S], BF16, tag=f"outT{r}", name=f"outT{r}") for r in range(NPAIR)]

        def mark(label):
            PHASE_LOG.append((label, nc._state.get_next_instruction_name()))

        with (
            tc.tile_pool(name="qx", bufs=2) as qx_pool,
            tc.tile_pool(name="kx", bufs=4) as kx_pool,
            tc.tile_pool(name="vx", bufs=2) as vx_pool,
            tc.tile_pool(name="pexp", bufs=26) as pexp_pool,
            tc.tile_pool(name="rc", bufs=8) as rc_pool,
            tc.tile_pool(name="n2", bufs=8) as n2_pool,
            tc.tile_pool(name="fo", bufs=10) as fo_pool,
            tc.tile_pool(name="foq", bufs=3) as foq_pool,
            tc.tile_pool(name="ps_sc", bufs=2, space="PSUM") as ps_sc,
            tc.tile_pool(name="ps_acc", bufs=2, space="PSUM") as ps_acc,
            tc.tile_pool(name="ps_ms", bufs=2, space="PSUM") as ps_ms,
        ):
            # ---------- emitters ----------
            def emit_q_dmas(ib_, split=False):
                """Load the [D, IB] Q slice; optionally as two adjacent
                column-half DMAs so the first qproj half starts sooner."""
                t = qx_pool.tile([128, DCH, IB], BF16, tag="qx", name="qx")
                qsrc = qt_d.rearrange("(c p) s -> p c s", p=128)
                if split:
                    for h in range(2):
                        i0 = IB * ib_ + 512 * h
                        nc.sync.dma_start(
                            out=t[:, :, 512 * h : 512 * (h + 1)],
                            in_=qsrc[:, :, i0 : i0 + 512],
                        )
                else:
                    nc.sync.dma_start(
                        out=t[:], in_=qsrc[:, :, IB * ib_ : IB * (ib_ + 1)]
                    )
                return [t[:, d, :] for d in range(DCH)]

            def emit_kv_dmas(sblk, dram, pool, tag):
                t = pool.tile([128, DCH, 512], BF16, tag=tag, name=tag)
                nc.sync.dma_start(
                    out=t[:],
                    in_=dram.rearrange("(c p) s -> p c s", p=128)[
                        :, :, 512 * sblk : 512 * (sblk + 1)
                    ],
                )
                return [t[:, d, :] for d in range(DCH)]

            def emit_qproj_half(ib_, r, half, qx):
                """One 512-col half of the Q projection for pair r."""
                mark(f"qproj{ib_}r{r}h{half}")
                i0 = IB * ib_ + 512 * half
                ps_q = ps_ms.tile([128, 512], F32, tag="ms", name="ps_q")
                for d in range(DCH):
                    w_st = wq_sb[:, CW * d + 128 * r : CW * d + 128 * (r + 1)]
                    nc.tensor.matmul(
                        ps_q[:],
                        w_st,
                        qx[d][:, 512 * half : 512 * (half + 1)],
                        start=(d == 0),
                        stop=(d == DCH - 1),
                    )
                nc.vector.tensor_scalar_add(
                    qhT[r][:, i0 : i0 + 512], ps_q[:], bq_sb[:, r : r + 1]
                )

            def emit_kproj_pair(sblk, r, kx):
                mark(f"kproj{sblk}r{r}")
                ps_kb = ps_ms.tile([128, 512], F32, tag="ms", name="ps_kb")
                for d in range(DCH):
                    w_st = wk_sb[:, CW * d + 128 * r : CW * d + 128 * (r + 1)]
                    nc.tensor.matmul(
                        ps_kb[:],
                        w_st,
                        kx[d][:],
                        start=(d == 0),
                        stop=(d == DCH - 1),
                    )
                nc.vector.tensor_scalar_add(
                    khT[r][:, 512 * sblk : 512 * (sblk + 1)],
                    ps_kb[:],
                    bk_sb[:, r : r + 1],
                )

            def emit_vproj_jpair(j0, vx):
                """Project V for j-chunks j0, j0+1: two 256-col accumulation
                groups in one PSUM bank (the first group's start=True zeroes
                the whole bank region), then scatter into vhe."""
                mark(f"vproj{j0}")
                reg = ps_ms.tile([128, 512], F32, tag="ms", name="ps_vh")
                for jh in range(2):
                    j = j0 + jh
                    sub = reg[:, 256 * jh : 256 * (jh + 1)]
                    for d in range(DCH):
                        nc.tensor.matmul(
                            sub,
                            vx[d][:, 128 * (j % 4) : 128 * (j % 4 + 1)],
                            wv_sb[:, CW * d : CW * (d + 1)],
                            start=(d == 0 and jh == 0),
                            stop=(jh == 1 and d == DCH - 1),
                            skip_group_check=True,
                        )
                for jh in range(2):
                    j = j0 + jh
                    sub = reg[:, 256 * jh : 256 * (jh + 1)]
                    for r in range(NPAIR):
                        dst = vhe[r][:, 130 * j : 130 * j + 130]
                        nc.vector.tensor_copy(
                            dst.rearrange("p (b e) -> p b e", e=65)[:, :, 0:64],
                            sub[:, 128 * r : 128 * (r + 1)]
                            .rearrange("p (b e) -> p b e", e=64),
                        )

            # pending normalize state: (accA, accB, h, ib)
            pending_norm_box = [None]
            norm2_box = [[None] * 8, [None] * 8]  # per ib: 8 n2 tiles

            def emit_norm(pend, ics=range(8), transpose=True):
                accA, accB, h, ib_ = pend
                r, q = h // 2, h % 2
                mark(f"norm_h{h}i{ib_}")
                with nc.named_scope(f"norm_h{h}i{ib_}"):
                    for bk_i, acc in ((0, accA), (1, accB)):
                        bank_ics = [ic for ic in ics if (ic // 4) == bk_i]
                        if not bank_ics:
                            continue
                        rc = rc_pool.tile([128, 4], F32, tag="rc", name="rc")
                        nc.vector.reciprocal(
                            rc[:].rearrange("p (c w) -> p c w", w=1),
                            acc[:].rearrange("p (c w) -> p c w", w=128)[:, :, 64:65],
                        )
                        for ic in bank_ics:
                            off = 128 * (ic % 4)
                            if q == 0:
                                n2 = n2_pool.tile([128, 128], BF16, tag="n2", name="n2")
                                norm2_box[ib_][ic] = n2
                            else:
                                n2 = norm2_box[ib_][ic]
                            nc.vector.tensor_scalar_mul(
                                n2[:, 64 * q : 64 * (q + 1)],
                                acc[:, off : off + 64],
                                rc[:, (ic % 4) : (ic % 4) + 1],
                            )
                        if q == 1 and transpose:
                            tp = ps_ms.tile([128, 1024], BF16, tag="ms", name="tp")
                            for k, ic in enumerate(bank_ics):
                                nc.tensor.transpose(
                                    tp[:, 128 * k : 128 * (k + 1)],
                                    norm2_box[ib_][ic][:],
                                    ident_sb[:],
                                )
                            i0 = IB * ib_ + 128 * bank_ics[0]
                            nc.vector.tensor_scalar_add(
                                outT[r][:, i0 : i0 + 128 * len(bank_ics)],
                                tp[:, 0 : 128 * len(bank_ics)],
                                bv_sb[:, r : r + 1],
                            )

            def emit_final_half(cc, ib_, f, i4, copy_eng=None):
                """Partial final projection for pair cc only (host sums)."""
                mark(f"final{ib_}c{cc}f{f}i{i4}")
                i0 = IB * ib_ + 512 * i4
                pf = ps_ms.tile([128, 512], F32, tag="ms", name="pf")
                nc.tensor.matmul(
                    pf[:],
                    wf_sb[:, D * cc + 128 * f : D * cc + 128 * (f + 1)],
                    outT[cc][:, i0 : i0 + 512],
                    start=True,
                    stop=True,
                )
                fo = fo_pool.tile([128, 512], BF16, tag="fo", name="fo")
                if copy_eng == "act":
                    nc.scalar.copy(fo[:], pf[:])
                else:
                    nc.vector.tensor_copy(fo[:], pf[:])
                nc.sync.dma_start(
                    out=pt_d[cc][128 * f : 128 * (f + 1), i0 : i0 + 512], in_=fo[:]
                )

            def emit_pv(r, q, pexp, jc, accA, accB):
                vmov = vhe[r][:, 130 * jc + 65 * q : 130 * jc + 65 * (q + 1)]
                for ic in range(8):
                    tgt = accA if ic < 4 else accB
                    off = 128 * (ic % 4)
                    nc.tensor.matmul(
                        tgt[:, off : off + 65],
                        pexp[:, 128 * ic : 128 * (ic + 1)],
                        vmov,
                        start=(jc == 0 and ic % 4 == 0),
                        stop=(jc == JCH - 1),
                        skip_group_check=True,
                    )

            def emit_scores_exp(h, ib_, jc):
                """Scores + exp for head h; returns the pexp tile."""
                r, q = h // 2, h % 2
                mark(f"attn_h{h}i{ib_}jc{jc}")
                qs = slice(64 * q, 64 * (q + 1))
                s_ps = ps_sc.tile([128, IB], F32, tag="sc", name="s_ps")
                for k in range(IB // 512):
                    nc.tensor.matmul(
                        s_ps[:, 512 * k : 512 * (k + 1)],
                        khT[r][qs, 128 * jc : 128 * (jc + 1)],
                        qhT[r][qs, IB * ib_ + 512 * k : IB * ib_ + 512 * (k + 1)],
                        start=True,
                        stop=True,
                    )
                pexp = pexp_pool.tile([128, IB], BF16, tag="pexp", name="pexp")
                nc.scalar.activation(pexp[:], s_ps[:], AF.Exp, scale=INV_SQRT_DK)
                return pexp

            PVLAG = 6

            def emit_attn_jc(h, ib_, jc, accA, accB, prev_box):
                """Emit scores[jc] and exp[jc]; emit PV[jc-PVLAG] (software-
                pipelined so PV inputs are always long-ready)."""
                r, q = h // 2, h % 2
                qs = slice(64 * q, 64 * (q + 1))
                s_ps = ps_sc.tile([128, IB], F32, tag="sc", name="s_ps")
                mark(f"attn_h{h}i{ib_}jc{jc}")
                for k in range(IB // 512):
                    nc.tensor.matmul(
                        s_ps[:, 512 * k : 512 * (k + 1)],
                        khT[r][qs, 128 * jc : 128 * (jc + 1)],
                        qhT[r][qs, IB * ib_ + 512 * k : IB * ib_ + 512 * (k + 1)],
                        start=True,
                        stop=True,
                    )
                if len(prev_box) >= PVLAG:
                    emit_pv(r, q, *prev_box.pop(0), accA, accB)
                pexp = pexp_pool.tile([128, IB], BF16, tag="pexp", name="pexp")
                nc.scalar.activation(pexp[:], s_ps[:], AF.Exp, scale=INV_SQRT_DK)
                prev_box.append((pexp, jc))

            def emit_attention(h, ib_, deferred, defer_slots=(3, 5, 7, 9, 11, 13)):
                """Full attention for head h on i-block ib_. Emits the PREVIOUS
                head's normalize after jc 1 (its acc banks drain early)."""
                with nc.named_scope(f"attn{ib_}h{h}"):
                    accA = ps_acc.tile([128, 512], F32, tag="acc", name="accA")
                    accB = ps_acc.tile([128, 512], F32, tag="acc", name="accB")
                    prev_box = []
                    for jc in range(JCH):
                        emit_attn_jc(h, ib_, jc, accA, accB, prev_box)
                        if jc == 1 and pending_norm_box[0] is not None:
                            emit_norm(pending_norm_box[0])
                            pending_norm_box[0] = None
                        if jc in defer_slots and deferred:
                            deferred.pop(0)()
                    while prev_box:
                        emit_pv(h // 2, h % 2, *prev_box.pop(0), accA, accB)
                    pending_norm_box[0] = (accA, accB, h, ib_)
                while deferred:
                    deferred.pop(0)()

            # ---------- schedule ----------
            # prologue DMAs ordered so the chase's first consumers unblock in
            # the order the PE needs them: wk+K0 (kproj), wv+V0 (vproj),
            # wq+Q0 (qproj), biases/ones interleaved early (tiny)
            load_w(wk_sb, wk_d)
            kx_next = emit_kv_dmas(0, kt_d, kx_pool, "kx")
            load_w(wv_sb, wv_d)
            vx_next = emit_kv_dmas(0, vt_d, vx_pool, "vx")
            load_w(wq_sb, wq_d)
            qx0 = emit_q_dmas(0)
            load_b(bk_sb, bk_d)
            load_b(bq_sb, bq_d)
            load_b(bv_sb, bv_d)
            nc.sync.dma_start(out=ones128[:], in_=onesr_d)
            nc.sync.dma_start(out=ones32[:], in_=ones_d)
            nc.sync.dma_start(out=ident_sb[:], in_=ident_d)
            for r in range(NPAIR):
                nc.vector.tensor_copy(
                    vhe[r][:].rearrange("p (c w) -> p c w", w=65)[:, :, 64:65],
                    ones32[:],
                )

            # chase: per 512-j block: K/V dma (next), r0 K proj + V proj,
            # h0 attention. kproj r1 and qproj r1 are deferred into the
            # h1/h2 attention windows (ACT-bound there, PE-bound here).
            kx_all = [None] * NSB
            pexp_h1 = [None] * JCH
            with nc.named_scope("chase"):
                accA0 = ps_acc.tile([128, 512], F32, tag="acc", name="accA")
                accB0 = ps_acc.tile([128, 512], F32, tag="acc", name="accB")
                prev0 = []

                def chase_jc(jc):
                    emit_attn_jc(0, 0, jc, accA0, accB0, prev0)
                    pexp_h1[jc] = emit_scores_exp(1, 0, jc)

                for sblk in range(NSB):
                    kx, vx = kx_next, vx_next
                    kx_all[sblk] = kx
                    if sblk + 1 < NSB:
                        kx_next = emit_kv_dmas(sblk + 1, kt_d, kx_pool, "kx")
                        vx_next = emit_kv_dmas(sblk + 1, vt_d, vx_pool, "vx")
                    if sblk == 0:
                        emit_kproj_pair(sblk, 0, kx)
                        emit_vproj_jpair(0, vx)
                        emit_vproj_jpair(2, vx)
                        for half in range(2):
                            emit_qproj_half(0, 0, half, qx0)
                        chase_jc(0)
                        chase_jc(1)
                        chase_jc(2)
                        chase_jc(3)
                        emit_kproj_pair(0, 1, kx)
                    else:
                        emit_kproj_pair(sblk, 0, kx)
                        emit_vproj_jpair(4 * sblk, vx)
                        chase_jc(4 * sblk + 0)
                        chase_jc(4 * sblk + 1)
                        emit_vproj_jpair(4 * sblk + 2, vx)
                        chase_jc(4 * sblk + 2)
                        chase_jc(4 * sblk + 3)
                    if sblk == 2:
                        for half in range(2):
                            emit_qproj_half(0, 1, half, qx0)
                while prev0:
                    emit_pv(0, 0, *prev0.pop(0), accA0, accB0)
                pending_norm_box[0] = (accA0, accB0, 0, 0)

            # h0 norm releases the acc banks for h1's deferred PV burst,
            # which rides inside h2's unit so ACT never drains
            emit_norm(pending_norm_box[0])
            pending_norm_box[0] = None
            acc1_box = [None]

            def burst_half(lo, hi):
                with nc.named_scope("h1burst"):
                    if acc1_box[0] is None:
                        acc1_box[0] = (
                            ps_acc.tile([128, 512], F32, tag="acc", name="accA"),
                            ps_acc.tile([128, 512], F32, tag="acc", name="accB"),
                        )
                    accA1, accB1 = acc1_box[0]
                    for jc in range(lo, hi):
                        emit_pv(0, 1, pexp_h1[jc], jc, accA1, accB1)
                    if hi == JCH:
                        emit_norm((accA1, accB1, 1, 0))

            # steady phase: remaining 7 head-blocks, ACT(exp)-bound; defer the
            # leftover projections and ib0's final tiles into their PE slack
            qx1 = emit_q_dmas(1)
            nc.sync.dma_start(
                out=wf_sb[:].rearrange("p (c f) -> p c f", c=2),
                in_=wf_d.rearrange("(c p) f -> p c f", p=128),
            )
            def finals(cc, ib_, lo, hi, pair=False):
                fs = [
                    (lambda f=f, i4=i4: emit_final_half(cc, ib_, f, i4))
                    for i4 in range(IB // 512)
                    for f in range(D // 128)
                ][lo:hi]
                if not pair:
                    return fs
                return [
                    (lambda a=fs[i], b=(fs[i + 1] if i + 1 < len(fs) else None): (
                        a(), b() if b else None
                    ))
                    for i in range(0, len(fs), 2)
                ]

            d_h2i0 = [
                (lambda: emit_kproj_pair(1, 1, kx_all[1])),
                (lambda: burst_half(0, 8)),
                (lambda: burst_half(8, JCH)),
                (lambda: emit_kproj_pair(2, 1, kx_all[2])),
                (lambda: emit_kproj_pair(3, 1, kx_all[3])),
            ]
            d_h3i0 = [
                (lambda half=half: emit_qproj_half(1, 0, half, qx1))
                for half in range(2)
            ] + finals(0, 0, 0, 6)
            d_h0i1 = finals(0, 0, 6, 16) + finals(1, 0, 0, 4)
            d_h1i1 = [
                (lambda half=half: emit_qproj_half(1, 1, half, qx1))
                for half in range(2)
            ] + finals(1, 0, 4, 14)
            d_h2i1 = finals(1, 0, 14, 16) + finals(0, 1, 0, 8)
            d_h3i1 = finals(0, 1, 8, 16)
            emit_attention(2, 0, d_h2i0, defer_slots=(0, 3, 6, 9, 12))
            emit_attention(3, 0, d_h3i0, defer_slots=(1, 5, 9, 10, 11, 12, 13, 14))
            emit_attention(0, 1, d_h0i1, defer_slots=tuple(range(2, 16)))
            emit_attention(1, 1, d_h1i1, defer_slots=(2, 6) + tuple(range(8, 15)))
            emit_attention(2, 1, d_h2i1, defer_slots=tuple(range(2, 14)))
            emit_attention(3, 1, d_h3i1, defer_slots=(2, 3, 4, 5, 6, 7, 8, 9))
            # tail: h3-ib1's normalize split per 512-i block, with the final
            # projection tiles for each block released as soon as its
            # transposes complete
            pend = pending_norm_box[0]
            pending_norm_box[0] = None
            with nc.named_scope("final1"):
                for i4 in range(2):
                    emit_norm(pend, ics=range(4 * i4, 4 * i4 + 4))
                    i0 = IB + 512 * i4
                    for quad in range(2):
                        foq = foq_pool.tile([128, 4, 512], BF16, tag="foq", name="foq")
                        # pair (sc banks) + two singles (idle acc banks):
                        # 4 psum buffers rotating so copies never pace PE
                        f0 = 4 * quad
                        mark(f"final1c1f{f0}i{i4}")
                        pf = ps_sc.tile([128, IB], F32, tag="sc", name="pfp")
                        for n in range(2):
                            nc.tensor.matmul(
                                pf[:, 512 * n : 512 * (n + 1)],
                                wf_sb[:, D + 128 * (f0 + n) : D + 128 * (f0 + n + 1)],
                                outT[1][:, i0 : i0 + 512],
                                start=True,
                                stop=True,
                            )
                        eng0 = nc.scalar.copy if quad % 2 == 0 else nc.vector.tensor_copy
                        eng0(foq[:, 0:2, :], pf[:].rearrange("p (a w) -> p a w", a=2))
                        for n in range(2, 4):
                            f = 4 * quad + n
                            mark(f"final1c1f{f}i{i4}")
                            pfs = ps_acc.tile([128, 512], F32, tag="acc", name="pfs")
                            nc.tensor.matmul(
                                pfs[:],
                                wf_sb[:, D + 128 * f : D + 128 * (f + 1)],
                                outT[1][:, i0 : i0 + 512],
                                start=True,
                                stop=True,
                            )
                            if n % 2:
                                nc.vector.tensor_copy(foq[:, n, :], pfs[:])
                            else:
                                nc.scalar.copy(foq[:, n, :], pfs[:])
                        if i4 == 1 and quad == 1:
                            for pr in range(2):
                                nc.sync.dma_start(
                                    out=pt_d[1]
                                    .rearrange("(f p) s -> p f s", p=128)[
                                        :,
                                        4 + 2 * pr : 6 + 2 * pr,
                                        i0 : i0 + 512,
                                    ],
                                    in_=foq[:, 2 * pr : 2 * pr + 2, :],
                                )
                        else:
                            nc.sync.dma_start(
                                out=pt_d[1]
                                .rearrange("(f p) s -> p f s", p=128)[
                                    :, 4 * quad : 4 * quad + 4, i0 : i0 + 512
                                ],
                                in_=foq[:],
                            )

    nc.compile()
    return nc


def _get_nc():
    if "nc" not in _CACHE:
        _CACHE["nc"] = _build()
    return _CACHE["nc"]


def kernel(Q, K, V, Wq, bq, Wk, bk, Wv, bv, Wf, bf):
    Q, K, V = np.asarray(Q), np.asarray(K), np.asarray(V)
    Wq, Wk, Wv, Wf = (np.asarray(a) for a in (Wq, Wk, Wv, Wf))
    bq, bk, bv, bf = (np.asarray(a) for a in (bq, bk, bv, bf))

    nc = _get_nc()

    def tobf(x):
        return np.ascontiguousarray(x.astype(NPBF16))

    qt = [tobf(Q[b].T) for b in range(B)]
    kt = [tobf(K[b].T) for b in range(B)]
    vt = [tobf(V[b].T) for b in range(B)]
    wq_g = [tobf(Wq[HPG * g : HPG * (g + 1)].transpose(1, 0, 2).reshape(D, CW)) for g in range(GPB)]
    wk_g = [tobf(Wk[HPG * g : HPG * (g + 1)].transpose(1, 0, 2).reshape(D, CW)) for g in range(GPB)]
    wv_g = [tobf(Wv[HPG * g : HPG * (g + 1)].transpose(1, 0, 2).reshape(D, CW)) for g in range(GPB)]
    wf_g = [tobf(Wf[CW * g : CW * (g + 1), :]) for g in range(GPB)]
    bq_g = [np.ascontiguousarray(bq[HPG * g : HPG * (g + 1)].reshape(CW), np.float32) for g in range(GPB)]
    bk_g = [np.ascontiguousarray(bk[HPG * g : HPG * (g + 1)].reshape(CW), np.float32) for g in range(GPB)]
    bv_g = [np.ascontiguousarray(bv[HPG * g : HPG * (g + 1)].reshape(CW), np.float32) for g in range(GPB)]

    ones_col = np.ones((128, 2 * JCH, 1), NPBF16)
    ones_row = np.ones((1, 128), NPBF16)
    ident = np.eye(128, dtype=NPBF16)
    in_maps = []
    for c in range(NCORES):
        b, g = c // GPB, c % GPB
        in_maps.append(
            {
                "qt": qt[b], "kt": kt[b], "vt": vt[b],
                "wq": wq_g[g], "wk": wk_g[g], "wv": wv_g[g], "wf": wf_g[g],
                "bq": bq_g[g], "bk": bk_g[g], "bv": bv_g[g],
                "ones32": ones_col, "ones_row": ones_row, "ident": ident,
            }
        )

    res = run_bass_kernel_spmd(nc, in_maps, list(range(NCORES)))

    out = np.empty((B, S, D), np.float32)
    bf32 = bf.astype(np.float32)
    for b in range(B):
        acc = None
        for g in range(GPB):
            r = res.results[GPB * b + g]
            part = r["pt0"].astype(np.float32) + r["pt1"].astype(np.float32)
            acc = part if acc is None else acc + part
        out[b] = acc.T + bf32
    return out


# revision 61
# speedup vs baseline: 1.0242x; 1.0002x over previous
"""Multi-head attention (B=2, S=2048, D=1024, H=16, DH=64) on 8 TRN2 cores.

Sharding: core c handles batch b = c//4 and head group g = c%4 (4 heads).
Per core, for its (b, g):
    qhT/khT = per-head-pair projections in transposed layout [e, s] (bf16),
    vhe = projected V in natural [j, e] layout with a ones column per head,
    S^T = Kh @ Qh^T per head (keys j on partitions),
    P^T = exp(S^T / sqrt(dk)) -> bf16 SBUF tiles [j, i],
    PV (transposed): stationary = P^T chunk [128 j, 128 i], moving =
        vhe [128 j, 65] -> acc[i, e|den] accumulated over j-chunks in PSUM.
        The ones column makes acc[:, 64] the softmax denominator, which sits
        on the partition (i) axis so normalization is a per-partition
        tensor_scalar multiply on DVE.
    norm2 [i, e-pair] tiles are DMA-transposed (XBAR) into outT [e, i],
    PT_partial = Wf^T-slice @ outT  -> partial final projection [D, S],
        DMA'd directly from PSUM to HBM.
Host: out[b] = (sum_g PT_partial).T + bf.

All matmul inputs are bf16 (1 PE cycle/row); PSUM accumulation is fp32.
Multiple PSUM accumulation groups share a bank: the bank's first matmul uses
start=True (which zeroes the whole 2KB region), later groups start on
start=False over the zeroed space.

Schedule: K/V stream and project per 512-key block with head-0 attention
chasing them; remaining heads run ACT(exp)-bound with Q-ib1 projection and
the final projection tiles deferred into their PE slack.
"""

import sys

sys.path.insert(0, "/opt/trn_rl_repo")

from contextlib import ExitStack

import ml_dtypes
import numpy as np

import concourse.mybir as mybir
import concourse.tile as tile
from concourse import bacc
from concourse.bass_utils import run_bass_kernel_spmd

B, S, D, H, DH = 2, 2048, 1024, 16, 64
NCORES = 8
GPB = 4  # head-group cores per batch
HPG = H // GPB  # heads per group (4)
CW = HPG * DH  # concat width per core (256)
NPAIR = HPG // 2  # head pairs per group (2)
DCH = D // 128  # d chunks (8)
JCH = S // 128  # key chunks (16)
NSB = S // 512  # 512-wide key stream blocks (4)
IB = 1024  # i-block width for attention
NIB = S // IB  # 2
F32 = mybir.dt.float32
BF16 = mybir.dt.bfloat16
AF = mybir.ActivationFunctionType
INV_SQRT_DK = 1.0 / np.sqrt(DH)
NPBF16 = ml_dtypes.bfloat16

_CACHE = {}
PHASE_LOG = []  # (label, next_instruction_name) markers recorded during build




def _build():
    nc = bacc.Bacc("TRN2", target_bir_lowering=False, debug=False, num_devices=NCORES)

    qt_d = nc.dram_tensor("qt", [D, S], BF16, kind="ExternalInput").ap()
    kt_d = nc.dram_tensor("kt", [D, S], BF16, kind="ExternalInput").ap()
    vt_d = nc.dram_tensor("vt", [D, S], BF16, kind="ExternalInput").ap()
    wq_d = nc.dram_tensor("wq", [D, CW], BF16, kind="ExternalInput").ap()
    wk_d = nc.dram_tensor("wk", [D, CW], BF16, kind="ExternalInput").ap()
    wv_d = nc.dram_tensor("wv", [D, CW], BF16, kind="ExternalInput").ap()
    wf_d = nc.dram_tensor("wf", [CW, D], BF16, kind="ExternalInput").ap()
    bq_d = nc.dram_tensor("bq", [CW], F32, kind="ExternalInput").ap()
    bk_d = nc.dram_tensor("bk", [CW], F32, kind="ExternalInput").ap()
    bv_d = nc.dram_tensor("bv", [CW], F32, kind="ExternalInput").ap()
    ones_d = nc.dram_tensor("ones32", [128, 2 * JCH, 1], BF16, kind="ExternalInput").ap()
    onesr_d = nc.dram_tensor("ones_row", [1, 128], BF16, kind="ExternalInput").ap()
    ident_d = nc.dram_tensor("ident", [128, 128], BF16, kind="ExternalInput").ap()
    pt_d = [
        nc.dram_tensor(f"pt{cc}", [D, S], BF16, kind="ExternalOutput").ap()
        for cc in range(2)
    ]

    with (
        tile.TileContext(nc) as tc,
        nc.allow_low_precision(reason="bf16 matmul pipeline is intentional"),
        ExitStack() as ctx,
    ):
        const = ctx.enter_context(tc.tile_pool(name="const", bufs=1))
        persist = ctx.enter_context(tc.tile_pool(name="persist", bufs=1))

        wq_sb = const.tile([128, DCH * CW], BF16, tag="wq")
        wk_sb = const.tile([128, DCH * CW], BF16, tag="wk")
        wv_sb = const.tile([128, DCH * CW], BF16, tag="wv")
        wf_sb = const.tile([128, 2 * D], BF16, tag="wf")
        bq_sb = const.tile([128, NPAIR], F32, tag="bq")
        bk_sb = const.tile([128, NPAIR], F32, tag="bk")
        bv_sb = const.tile([128, NPAIR], F32, tag="bv")
        ones128 = const.tile([1, 128], BF16, tag="ones")
        ident_sb = const.tile([128, 128], BF16, tag="ident")
        ones32 = const.tile([128, 2 * JCH, 1], BF16, tag="ones32")

        def load_w(w_sb, w_dram):
            nc.sync.dma_start(
                out=w_sb[:].rearrange("p (c e) -> p c e", c=DCH),
                in_=w_dram.rearrange("(c p) e -> p c e", p=128),
            )

        def load_b(b_sb, b_dram):
            nc.sync.dma_start(out=b_sb[:], in_=b_dram.rearrange("(r p) -> p r", p=128))

        qhT = [persist.tile([128, S], BF16, tag=f"qhT{r}", name=f"qhT{r}") for r in range(NPAIR)]
        khT = [persist.tile([128, S], BF16, tag=f"khT{r}", name=f"khT{r}") for r in range(NPAIR)]
        vhe = [persist.tile([128, JCH * 130], BF16, tag=f"vhe{r}", name=f"vhe{r}") for r in range(NPAIR)]
        outT = [persist.tile([128, S], BF16, tag=f"outT{r}", name=f"outT{r}") for r in range(NPAIR)]

        def mark(label):
            PHASE_LOG.append((label, nc._state.get_next_instruction_name()))

        with (
            tc.tile_pool(name="qx", bufs=2) as qx_pool,
            tc.tile_pool(name="kx", bufs=4) as kx_pool,
            tc.tile_pool(name="vx", bufs=2) as vx_pool,
            tc.tile_pool(name="pexp", bufs=26) as pexp_pool,
            tc.tile_pool(name="rc", bufs=8) as rc_pool,
            tc.tile_pool(name="n2", bufs=8) as n2_pool,
            tc.tile_pool(name="fo", bufs=10) as fo_pool,
            tc.tile_pool(name="foq", bufs=3) as foq_pool,
            tc.tile_pool(name="ps_sc", bufs=2, space="PSUM") as ps_sc,
            tc.tile_pool(name="ps_acc", bufs=2, space="PSUM") as ps_acc,
            tc.tile_pool(name="ps_ms", bufs=2, space="PSUM") as ps_ms,
        ):
            # ---------- emitters ----------
            def emit_q_dmas(ib_, split=False):
                """Load the [D, IB] Q slice; optionally as two adjacent
                column-half DMAs so the first qproj half starts sooner."""
                t = qx_pool.tile([128, DCH, IB], BF16, tag="qx", name="qx")
                qsrc = qt_d.rearrange("(c p) s -> p c s", p=128)
                if split:
                    for h in range(2):
                        i0 = IB * ib_ + 512 * h
                        nc.sync.dma_start(
                            out=t[:, :, 512 * h : 512 * (h + 1)],
                            in_=qsrc[:, :, i0 : i0 + 512],
                        )
                else:
                    nc.sync.dma_start(
                        out=t[:], in_=qsrc[:, :, IB * ib_ : IB * (ib_ + 1)]
                    )
                return [t[:, d, :] for d in range(DCH)]

            def emit_kv_dmas(sblk, dram, pool, tag):
                t = pool.tile([128, DCH, 512], BF16, tag=tag, name=tag)
                nc.sync.dma_start(
                    out=t[:],
                    in_=dram.rearrange("(c p) s -> p c s", p=128)[
                        :, :, 512 * sblk : 512 * (sblk + 1)
                    ],
                )
                return [t[:, d, :] for d in range(DCH)]

            def emit_qproj_half(ib_, r, half, qx):
                """One 512-col half of the Q projection for pair r."""
                mark(f"qproj{ib_}r{r}h{half}")
                i0 = IB * ib_ + 512 * half
                ps_q = ps_ms.tile([128, 512], F32, tag="ms", name="ps_q")
                for d in range(DCH):
                    w_st = wq_sb[:, CW * d + 128 * r : CW * d + 128 * (r + 1)]
                    nc.tensor.matmul(
                        ps_q[:],
                        w_st,
                        qx[d][:, 512 * half : 512 * (half + 1)],
                        start=(d == 0),
                        stop=(d == DCH - 1),
                    )
                nc.vector.tensor_scalar_add(
                    qhT[r][:, i0 : i0 + 512], ps_q[:], bq_sb[:, r : r + 1]
                )

            def emit_kproj_pair(sblk, r, kx):
                mark(f"kproj{sblk}r{r}")
                ps_kb = ps_ms.tile([128, 512], F32, tag="ms", name="ps_kb")
                for d in range(DCH):
                    w_st = wk_sb[:, CW * d + 128 * r : CW * d + 128 * (r + 1)]
                    nc.tensor.matmul(
                        ps_kb[:],
                        w_st,
                        kx[d][:],
                        start=(d == 0),
                        stop=(d == DCH - 1),
                    )
                nc.vector.tensor_scalar_add(
                    khT[r][:, 512 * sblk : 512 * (sblk + 1)],
                    ps_kb[:],
                    bk_sb[:, r : r + 1],
                )

            def emit_vproj_jpair(j0, vx):
                """Project V for j-chunks j0, j0+1: two 256-col accumulation
                groups in one PSUM bank (the first group's start=True zeroes
                the whole bank region), then scatter into vhe."""
                mark(f"vproj{j0}")
                reg = ps_ms.tile([128, 512], F32, tag="ms", name="ps_vh")
                for jh in range(2):
                    j = j0 + jh
                    sub = reg[:, 256 * jh : 256 * (jh + 1)]
                    for d in range(DCH):
                        nc.tensor.matmul(
                            sub,
                            vx[d][:, 128 * (j % 4) : 128 * (j % 4 + 1)],
                            wv_sb[:, CW * d : CW * (d + 1)],
                            start=(d == 0 and jh == 0),
                            stop=(jh == 1 and d == DCH - 1),
                            skip_group_check=True,
                        )
                for jh in range(2):
                    j = j0 + jh
                    sub = reg[:, 256 * jh : 256 * (jh + 1)]
                    for r in range(NPAIR):
                        dst = vhe[r][:, 130 * j : 130 * j + 130]
                        nc.vector.tensor_copy(
                            dst.rearrange("p (b e) -> p b e", e=65)[:, :, 0:64],
                            sub[:, 128 * r : 128 * (r + 1)]
                            .rearrange("p (b e) -> p b e", e=64),
                        )

            # pending normalize state: (accA, accB, h, ib)
            pending_norm_box = [None]
            norm2_box = [[None] * 8, [None] * 8]  # per ib: 8 n2 tiles

            def emit_norm(pend, ics=range(8), transpose=True):
                accA, accB, h, ib_ = pend
                r, q = h // 2, h % 2
                mark(f"norm_h{h}i{ib_}")
                with nc.named_scope(f"norm_h{h}i{ib_}"):
                    for bk_i, acc in ((0, accA), (1, accB)):
                        bank_ics = [ic for ic in ics if (ic // 4) == bk_i]
                        if not bank_ics:
                            continue
                        rc = rc_pool.tile([128, 4], F32, tag="rc", name="rc")
                        nc.vector.reciprocal(
                            rc[:].rearrange("p (c w) -> p c w", w=1),
                            acc[:].rearrange("p (c w) -> p c w", w=128)[:, :, 64:65],
                        )
                        for ic in bank_ics:
                            off = 128 * (ic % 4)
                            if q == 0:
                                n2 = n2_pool.tile([128, 128], BF16, tag="n2", name="n2")
                                norm2_box[ib_][ic] = n2
                            else:
                                n2 = norm2_box[ib_][ic]
                            nc.vector.tensor_scalar_mul(
                                n2[:, 64 * q : 64 * (q + 1)],
                                acc[:, off : off + 64],
                                rc[:, (ic % 4) : (ic % 4) + 1],
                            )
                        if q == 1 and transpose:
                            tp = ps_ms.tile([128, 1024], BF16, tag="ms", name="tp")
                            for k, ic in enumerate(bank_ics):
                                nc.tensor.transpose(
                                    tp[:, 128 * k : 128 * (k + 1)],
                                    norm2_box[ib_][ic][:],
                                    ident_sb[:],
                                )
                            i0 = IB * ib_ + 128 * bank_ics[0]
                            nc.vector.tensor_scalar_add(
                                outT[r][:, i0 : i0 + 128 * len(bank_ics)],
                                tp[:, 0 : 128 * len(bank_ics)],
                                bv_sb[:, r : r + 1],
                            )

            def emit_final_half(cc, ib_, f, i4, copy_eng=None):
                """Partial final projection for pair cc only (host sums)."""
                mark(f"final{ib_}c{cc}f{f}i{i4}")
                i0 = IB * ib_ + 512 * i4
                pf = ps_ms.tile([128, 512], F32, tag="ms", name="pf")
                nc.tensor.matmul(
                    pf[:],
                    wf_sb[:, D * cc + 128 * f : D * cc + 128 * (f + 1)],
                    outT[cc][:, i0 : i0 + 512],
                    start=True,
                    stop=True,
                )
                fo = fo_pool.tile([128, 512], BF16, tag="fo", name="fo")
                if copy_eng == "act":
                    nc.scalar.copy(fo[:], pf[:])
                else:
                    nc.vector.tensor_copy(fo[:], pf[:])
                nc.sync.dma_start(
                    out=pt_d[cc][128 * f : 128 * (f + 1), i0 : i0 + 512], in_=fo[:]
                )

            def emit_pv(r, q, pexp, jc, accA, accB):
                vmov = vhe[r][:, 130 * jc + 65 * q : 130 * jc + 65 * (q + 1)]
                for ic in range(8):
                    tgt = accA if ic < 4 else accB
                    off = 128 * (ic % 4)
                    nc.tensor.matmul(
                        tgt[:, off : off + 65],
                        pexp[:, 128 * ic : 128 * (ic + 1)],
                        vmov,
                        start=(jc == 0 and ic % 4 == 0),
                        stop=(jc == JCH - 1),
                        skip_group_check=True,
                    )

            def emit_scores_exp(h, ib_, jc):
                """Scores + exp for head h; returns the pexp tile."""
                r, q = h // 2, h % 2
                mark(f"attn_h{h}i{ib_}jc{jc}")
                qs = slice(64 * q, 64 * (q + 1))
                s_ps = ps_sc.tile([128, IB], F32, tag="sc", name="s_ps")
                for k in range(IB // 512):
                    nc.tensor.matmul(
                        s_ps[:, 512 * k : 512 * (k + 1)],
                        khT[r][qs, 128 * jc : 128 * (jc + 1)],
                        qhT[r][qs, IB * ib_ + 512 * k : IB * ib_ + 512 * (k + 1)],
                        start=True,
                        stop=True,
                    )
                pexp = pexp_pool.tile([128, IB], BF16, tag="pexp", name="pexp")
                nc.scalar.activation(pexp[:], s_ps[:], AF.Exp, scale=INV_SQRT_DK)
                return pexp

            PVLAG = 6

            def emit_attn_jc(h, ib_, jc, accA, accB, prev_box):
                """Emit scores[jc] and exp[jc]; emit PV[jc-PVLAG] (software-
                pipelined so PV inputs are always long-ready)."""
                r, q = h // 2, h % 2
                qs = slice(64 * q, 64 * (q + 1))
                s_ps = ps_sc.tile([128, IB], F32, tag="sc", name="s_ps")
                mark(f"attn_h{h}i{ib_}jc{jc}")
                for k in range(IB // 512):
                    nc.tensor.matmul(
                        s_ps[:, 512 * k : 512 * (k + 1)],
                        khT[r][qs, 128 * jc : 128 * (jc + 1)],
                        qhT[r][qs, IB * ib_ + 512 * k : IB * ib_ + 512 * (k + 1)],
                        start=True,
                        stop=True,
                    )
                if len(prev_box) >= PVLAG:
                    emit_pv(r, q, *prev_box.pop(0), accA, accB)
                pexp = pexp_pool.tile([128, IB], BF16, tag="pexp", name="pexp")
                nc.scalar.activation(pexp[:], s_ps[:], AF.Exp, scale=INV_SQRT_DK)
                prev_box.append((pexp, jc))

            def emit_attention(h, ib_, deferred, defer_slots=(3, 5, 7, 9, 11, 13)):
                """Full attention for head h on i-block ib_. Emits the PREVIOUS
                head's normalize after jc 1 (its acc banks drain early)."""
                with nc.named_scope(f"attn{ib_}h{h}"):
                    accA = ps_acc.tile([128, 512], F32, tag="acc", name="accA")
                    accB = ps_acc.tile([128, 512], F32, tag="acc", name="accB")
                    prev_box = []
                    for jc in range(JCH):
                        emit_attn_jc(h, ib_, jc, accA, accB, prev_box)
                        if jc == 1 and pending_norm_box[0] is not None:
                            emit_norm(pending_norm_box[0])
                            pending_norm_box[0] = None
                        if jc in defer_slots and deferred:
                            deferred.pop(0)()
                    while prev_box:
                        emit_pv(h // 2, h % 2, *prev_box.pop(0), accA, accB)
                    pending_norm_box[0] = (accA, accB, h, ib_)
                while deferred:
                    deferred.pop(0)()

            # ---------- schedule ----------
            # prologue DMAs ordered so the chase's first consumers unblock in
            # the order the PE needs them: wk+K0 (kproj), wv+V0 (vproj),
            # wq+Q0 (qproj), biases/ones interleaved early (tiny)
            load_w(wk_sb, wk_d)
            kx_next = emit_kv_dmas(0, kt_d, kx_pool, "kx")
            load_w(wv_sb, wv_d)
            vx_next = emit_kv_dmas(0, vt_d, vx_pool, "vx")
            load_w(wq_sb, wq_d)
            qx0 = emit_q_dmas(0)
            load_b(bk_sb, bk_d)
            load_b(bq_sb, bq_d)
            load_b(bv_sb, bv_d)
            nc.sync.dma_start(out=ones128[:], in_=onesr_d)
            nc.sync.dma_start(out=ones32[:], in_=ones_d)
            nc.sync.dma_start(out=ident_sb[:], in_=ident_d)
            for r in range(NPAIR):
                nc.vector.tensor_copy(
                    vhe[r][:].rearrange("p (c w) -> p c w", w=65)[:, :, 64:65],
                    ones32[:],
                )

            # chase: per 512-j block: K/V dma (next), r0 K proj + V proj,
            # h0 attention. kproj r1 and qproj r1 are deferred into the
            # h1/h2 attention windows (ACT-bound there, PE-bound here).
            kx_all = [None] * NSB
            pexp_h1 = [None] * JCH
            with nc.named_scope("chase"):
                accA0 = ps_acc.tile([128, 512], F32, tag="acc", name="accA")
                accB0 = ps_acc.tile([128, 512], F32, tag="acc", name="accB")
                prev0 = []

                def chase_jc(jc):
                    emit_attn_jc(0, 0, jc, accA0, accB0, prev0)
                    pexp_h1[jc] = emit_scores_exp(1, 0, jc)

                for sblk in range(NSB):
                    kx, vx = kx_next, vx_next
                    kx_all[sblk] = kx
                    if sblk + 1 < NSB:
                        kx_next = emit_kv_dmas(sblk + 1, kt_d, kx_pool, "kx")
                        vx_next = emit_kv_dmas(sblk + 1, vt_d, vx_pool, "vx")
                    if sblk == 0:
                        emit_kproj_pair(sblk, 0, kx)
                        emit_vproj_jpair(0, vx)
                        emit_vproj_jpair(2, vx)
                        for half in range(2):
                            emit_qproj_half(0, 0, half, qx0)
                        chase_jc(0)
                        chase_jc(1)
                        chase_jc(2)
                        chase_jc(3)
                        emit_kproj_pair(0, 1, kx)
                    else:
                        emit_kproj_pair(sblk, 0, kx)
                        emit_vproj_jpair(4 * sblk, vx)
                        chase_jc(4 * sblk + 0)
                        chase_jc(4 * sblk + 1)
                        emit_vproj_jpair(4 * sblk + 2, vx)
                        chase_jc(4 * sblk + 2)
                        chase_jc(4 * sblk + 3)
                    if sblk == 2:
                        for half in range(2):
                            emit_qproj_half(0, 1, half, qx0)
                while prev0:
                    emit_pv(0, 0, *prev0.pop(0), accA0, accB0)
                pending_norm_box[0] = (accA0, accB0, 0, 0)

            # h0 norm releases the acc banks for h1's deferred PV burst,
            # which rides inside h2's unit so ACT never drains
            emit_norm(pending_norm_box[0])
            pending_norm_box[0] = None
            acc1_box = [None]

            def burst_half(lo, hi):
                with nc.named_scope("h1burst"):
                    if acc1_box[0] is None:
                        acc1_box[0] = (
                            ps_acc.tile([128, 512], F32, tag="acc", name="accA"),
                            ps_acc.tile([128, 512], F32, tag="acc", name="accB"),
                        )
                    accA1, accB1 = acc1_box[0]
                    for jc in range(lo, hi):
                        emit_pv(0, 1, pexp_h1[jc], jc, accA1, accB1)
                    if hi == JCH:
                        emit_norm((accA1, accB1, 1, 0))

            # steady phase: remaining 7 head-blocks, ACT(exp)-bound; defer the
            # leftover projections and ib0's final tiles into their PE slack
            qx1 = emit_q_dmas(1)
            nc.sync.dma_start(
                out=wf_sb[:].rearrange("p (c f) -> p c f", c=2),
                in_=wf_d.rearrange("(c p) f -> p c f", p=128),
            )
            def finals(cc, ib_, lo, hi, pair=False):
                fs = [
                    (lambda f=f, i4=i4: emit_final_half(cc, ib_, f, i4))
                    for i4 in range(IB // 512)
                    for f in range(D // 128)
                ][lo:hi]
                if not pair:
                    return fs
                return [
                    (lambda a=fs[i], b=(fs[i + 1] if i + 1 < len(fs) else None): (
                        a(), b() if b else None
                    ))
                    for i in range(0, len(fs), 2)
                ]

            d_h2i0 = [
                (lambda: emit_kproj_pair(1, 1, kx_all[1])),
                (lambda: burst_half(0, 8)),
                (lambda: burst_half(8, JCH)),
                (lambda: emit_kproj_pair(2, 1, kx_all[2])),
                (lambda: emit_kproj_pair(3, 1, kx_all[3])),
            ]
            d_h3i0 = [
                (lambda half=half: emit_qproj_half(1, 0, half, qx1))
                for half in range(2)
            ] + finals(0, 0, 0, 6)
            d_h0i1 = finals(0, 0, 6, 16) + finals(1, 0, 0, 4)
            d_h1i1 = [
                (lambda half=half: emit_qproj_half(1, 1, half, qx1))
                for half in range(2)
            ] + finals(1, 0, 4, 14)
            d_h2i1 = finals(1, 0, 14, 16) + finals(0, 1, 0, 8)
            d_h3i1 = finals(0, 1, 8, 16)
            emit_attention(2, 0, d_h2i0, defer_slots=(0, 3, 6, 9, 12))
            emit_attention(3, 0, d_h3i0, defer_slots=(1, 5, 9, 10, 11, 12, 13, 14))
            emit_attention(0, 1, d_h0i1, defer_slots=tuple(range(2, 16)))
            emit_attention(1, 1, d_h1i1, defer_slots=(2, 6) + tuple(range(8, 15)))
            emit_attention(2, 1, d_h2i1, defer_slots=tuple(range(2, 14)))
            emit_attention(3, 1, d_h3i1, defer_slots=(2, 3, 4, 5, 6, 7, 8, 9))
            # tail: h3-ib1's normalize split per 512-i block, with the final
            # projection tiles for each block released as soon as its
            # transposes complete
            pend = pending_norm_box[0]
            pending_norm_box[0] = None
            with nc.named_scope("final1"):
                for i4 in range(2):
                    emit_norm(pend, ics=range(4 * i4, 4 * i4 + 4))
                    i0 = IB + 512 * i4
                    for quad in range(2):
                        foq = foq_pool.tile([128, 4, 512], BF16, tag="foq", name="foq")
                        # pair (sc banks) + two singles (idle acc banks):
                        # 4 psum buffers rotating so copies never pace PE
                        f0 = 4 * quad
                        mark(f"final1c1f{f0}i{i4}")
                        pf = ps_sc.tile([128, IB], F32, tag="sc", name="pfp")
                        for n in range(2):
                            nc.tensor.matmul(
                                pf[:, 512 * n : 512 * (n + 1)],
                                wf_sb[:, D + 128 * (f0 + n) : D + 128 * (f0 + n + 1)],
                                outT[1][:, i0 : i0 + 512],
                                start=True,
                                stop=True,
                            )
                        # split the pair copy across DVE and ACT so the
                        # pair tile frees in half the time
                        nc.vector.tensor_copy(foq[:, 0, :], pf[:, 0:512])
                        nc.scalar.copy(foq[:, 1, :], pf[:, 512:1024])
                        for n in range(2, 4):
                            f = 4 * quad + n
                            mark(f"final1c1f{f}i{i4}")
                            pfs = ps_acc.tile([128, 512], F32, tag="acc", name="pfs")
                            nc.tensor.matmul(
                                pfs[:],
                                wf_sb[:, D + 128 * f : D + 128 * (f + 1)],
                                outT[1][:, i0 : i0 + 512],
                                start=True,
                                stop=True,
                            )
                            if n % 2:
                                nc.vector.tensor_copy(foq[:, n, :], pfs[:])
                            else:
                                nc.scalar.copy(foq[:, n, :], pfs[:])
                        if i4 == 1 and quad == 1:
                            for pr in range(2):
                                nc.sync.dma_start(
                                    out=pt_d[1]
                                    .rearrange("(f p) s -> p f s", p=128)[
                                        :,
                                        4 + 2 * pr : 6 + 2 * pr,
                                        i0 : i0 + 512,
                                    ],
                                    in_=foq[:, 2 * pr : 2 * pr + 2, :],
                                )
                        else:
                            nc.sync.dma_start(
                                out=pt_d[1]
                                .rearrange("(f p) s -> p f s", p=128)[
                                    :, 4 * quad : 4 * quad + 4, i0 : i0 + 512
                                ],
                                in_=foq[:],
                            )

    nc.compile()
    return nc


def _get_nc():
    if "nc" not in _CACHE:
        _CACHE["nc"] = _build()
    return _CACHE["nc"]


def kernel(Q, K, V, Wq, bq, Wk, bk, Wv, bv, Wf, bf):
    Q, K, V = np.asarray(Q), np.asarray(K), np.asarray(V)
    Wq, Wk, Wv, Wf = (np.asarray(a) for a in (Wq, Wk, Wv, Wf))
    bq, bk, bv, bf = (np.asarray(a) for a in (bq, bk, bv, bf))

    nc = _get_nc()

    def tobf(x):
        return np.ascontiguousarray(x.astype(NPBF16))

    qt = [tobf(Q[b].T) for b in range(B)]
    kt = [tobf(K[b].T) for b in range(B)]
    vt = [tobf(V[b].T) for b in range(B)]
    wq_g = [tobf(Wq[HPG * g : HPG * (g + 1)].transpose(1, 0, 2).reshape(D, CW)) for g in range(GPB)]
    wk_g = [tobf(Wk[HPG * g : HPG * (g + 1)].transpose(1, 0, 2).reshape(D, CW)) for g in range(GPB)]
    wv_g = [tobf(Wv[HPG * g : HPG * (g + 1)].transpose(1, 0, 2).reshape(D, CW)) for g in range(GPB)]
    wf_g = [tobf(Wf[CW * g : CW * (g + 1), :]) for g in range(GPB)]
    bq_g = [np.ascontiguousarray(bq[HPG * g : HPG * (g + 1)].reshape(CW), np.float32) for g in range(GPB)]
    bk_g = [np.ascontiguousarray(bk[HPG * g : HPG * (g + 1)].reshape(CW), np.float32) for g in range(GPB)]
    bv_g = [np.ascontiguousarray(bv[HPG * g : HPG * (g + 1)].reshape(CW), np.float32) for g in range(GPB)]

    ones_col = np.ones((128, 2 * JCH, 1), NPBF16)
    ones_row = np.ones((1, 128), NPBF16)
    ident = np.eye(128, dtype=NPBF16)
    in_maps = []
    for c in range(NCORES):
        b, g = c // GPB, c % GPB
        in_maps.append(
            {
                "qt": qt[b], "kt": kt[b], "vt": vt[b],
                "wq": wq_g[g], "wk": wk_g[g], "wv": wv_g[g], "wf": wf_g[g],
                "bq": bq_g[g], "bk": bk_g[g], "bv": bv_g[g],
                "ones32": ones_col, "ones_row": ones_row, "ident": ident,
            }
        )

    res = run_bass_kernel_spmd(nc, in_maps, list(range(NCORES)))

    out = np.empty((B, S, D), np.float32)
    bf32 = bf.astype(np.float32)
    for b in range(B):
        acc = None
        for g in range(GPB):
            r = res.results[GPB * b + g]
            part = r["pt0"].astype(np.float32) + r["pt1"].astype(np.float32)
            acc = part if acc is None else acc + part
        out[b] = acc.T + bf32
    return out
